# revision 1
# baseline (speedup 1.0000x reference)
"""Multi-head causal attention with interleaved RoPE on 8 Trainium2 cores.

nn_MultiHeadAttention: x[4,2048,1024], W_qkv[3072,1024], W_o[1024,1024],
16 heads x d_k=64, interleaved RoPE, causal softmax.

Sharding: core c = 2*b + g handles batch b (of 4) and head-group g (of 2,
8 heads each). Each core computes a full-width partial output for its batch
(o_heads @ W_o[:, group-cols]); the host sums the two partials per batch
(the "all-reduce after o_proj", done on host at gather time).

Device strategy (per core):
 - host passes x[b] transposed (xT [1024,2048]) and W slices transposed, with
   q/k rows permuted even-first so interleaved RoPE becomes rotate-half.
 - fp32r matmuls everywhere (1 cyc/row vs 4 for fp32 at moving dim >= 256).
 - QKV proj on PE, out q^T/k^T in [head_dim, seq] layout; RoPE applied with
   a gpsimd 32-row swap + DVE mul/mul/add against host-built cos/sin tables
   (sign of sin baked into the table rows).
 - scores computed transposed: S^T[k,q] = k_rot . q_rot per head; exp on ACT
   (1/sqrt(dk) fused into the activation scale; no max-subtraction needed:
   scores are O(15) max, fp32-safe); causal = block skipping + one additive
   -1e30 mask on the 128x128 diagonal block before exp.
 - PV with lhsT = [v | ones]: the softmax denominator falls out as row 64 of
   the PSUM accumulator; normalize after PV, directly producing o^T which is
   exactly the lhsT that o_proj needs. No transposes of P or o anywhere.
"""

import numpy as np
from contextlib import ExitStack

NUM_HEADS = 16
D_K = 64
THETA = 10000.0
BS, S, D = 4, 2048, 1024
N_CORES = 8
HPC = NUM_HEADS // 2          # heads per core = 8
DG = HPC * D_K                # per-core head width = 512
QT2 = 1024                    # q tile (2 PSUM banks)

USE_F32R = True

_compiled = None


def _build_program(stop_after=None):
    import concourse.bass as bass
    import concourse.mybir as mybir
    import concourse.tile as tile
    from concourse import bacc

    F32 = mybir.dt.float32
    FR = mybir.dt.float32r if USE_F32R else mybir.dt.float32
    AF = mybir.ActivationFunctionType

    nc = bacc.Bacc("TRN2", target_bir_lowering=False, debug=False,
                   num_devices=N_CORES)

    xt_d = nc.dram_tensor("xt", [D, S], FR, kind="ExternalInput")
    wqkvt_d = nc.dram_tensor("wqkvt", [D, 3 * DG], FR, kind="ExternalInput")
    wot_d = nc.dram_tensor("wot", [DG, D], FR, kind="ExternalInput")
    perm_d = nc.dram_tensor("perm", [128, 128], FR, kind="ExternalInput")
    cos_d = nc.dram_tensor("cost", [128, S], F32, kind="ExternalInput")
    sin_d = nc.dram_tensor("sint", [128, S], F32, kind="ExternalInput")
    out_d = nc.dram_tensor("out", [S, D], F32, kind="ExternalOutput")

    n_sb = S // 128           # 16 s-blocks
    n_st = S // 512           # 4 s-tiles
    n_db = D // 128           # 8 d-blocks
    inv_sqrt_dk = 1.0 / float(np.sqrt(D_K))

    def load_xt_strip(pool, st):
        """One gpsimd DMA (fp32 -> fp32r cast): [128, 8*512] strip tile where
        cols db*512:(db+1)*512 hold xT[db*128:(db+1)*128, st*512:(st+1)*512]."""
        t = pool.tile([128, n_db * 512], FR, tag="xts", name="xts")
        src = xt_d.ap().rearrange("(db p) (st s) -> p db (st s)",
                                  p=128, st=n_st)
        nc.sync.dma_start(t[:].rearrange("p (db s) -> p db s", db=n_db),
                          src[:, :, st * 512:(st + 1) * 512])
        return t

    with tile.TileContext(nc) as tc, ExitStack() as octx:
        OP = octx.enter_context
        # ---------- persistent pools (whole kernel) ----------
        qk_p = OP(tc.tile_pool(name="qk", bufs=1))

        # q_rot/k_rot: [512 e, 2048 s] as 4 tiles of [128, S] each
        qrot = [qk_p.tile([128, S], FR, tag=f"qrot{i}", name=f"qrot{i}")
                for i in range(4)]
        krot = [qk_p.tile([128, S], FR, tag=f"krot{i}", name=f"krot{i}")
                for i in range(4)]
        wot_p = OP(tc.tile_pool(name="wot", bufs=1))
        wot = [wot_p.tile([128, D], FR, tag=f"wot{i}", name=f"wott{i}")
               for i in range(4)]
        const_p = OP(tc.tile_pool(name="amisc", bufs=1))
        # multiplicative causal mask for the S^T diagonal block:
        # 1 where k <= q, 0 where k > q
        dmask = const_p.tile([128, 128], F32)
        nc.gpsimd.memset(dmask[:], 1.0)
        nc.gpsimd.affine_select(
            out=dmask[:], in_=dmask[:],
            compare_op=mybir.AluOpType.is_ge, fill=0.0, base=0,
            pattern=[[1, 128]], channel_multiplier=-1,
        )

        # ============ Phase P1: q/k projection + RoPE ============
        with ExitStack() as ctx:
            P = ctx.enter_context
            cs_p = P(tc.tile_pool(name="cs", bufs=1))
            xt_p = P(tc.tile_pool(name="xtp", bufs=2))
            w_p = P(tc.tile_pool(name="w", bufs=1))
            tmp_p = P(tc.tile_pool(name="tmp", bufs=5))
            rot_p = P(tc.tile_pool(name="rot", bufs=4))
            pp1 = P(tc.tile_pool(name="pp1", bufs=8, space="PSUM"))

            # strip 0 first: the very first matmul needs it
            xts_next = load_xt_strip(xt_p, 0)
            # W_qk^T resident as separate lo/hi tiles (e 0..511 / 512..1023)
            # so early eb blocks depend only on the lo DMAs
            wqk_lo = [w_p.tile([128, DG], FR, tag=f"wqkl{i}", name=f"wqkl{i}")
                      for i in range(n_db)]
            wqk_hi = [w_p.tile([128, DG], FR, tag=f"wqkh{i}", name=f"wqkh{i}")
                      for i in range(n_db)]
            for db in range(n_db):
                nc.sync.dma_start(
                    wqk_lo[db][:],
                    wqkvt_d.ap()[db * 128:(db + 1) * 128, 0:512])
            perm_t = cs_p.tile([128, 128], FR, name="perm_t")
            nc.scalar.dma_start(perm_t[:], perm_d.ap())
            cos_t = cs_p.tile([128, S], F32)
            sin_t = cs_p.tile([128, S], F32)
            nc.scalar.dma_start(cos_t[:], cos_d.ap())
            nc.scalar.dma_start(sin_t[:], sin_d.ap())
            for db in range(n_db):
                nc.sync.dma_start(
                    wqk_hi[db][:],
                    wqkvt_d.ap()[db * 128:(db + 1) * 128, 512:1024])

            def wqk_slice(db, eb):
                if eb < 4:
                    return wqk_lo[db][:, eb * 128:(eb + 1) * 128]
                return wqk_hi[db][:, (eb - 4) * 128:(eb - 3) * 128]

            def rope_phase2(state):
                """swap-matmul + t2 + add for a previous block (lag-1 so the
                perm matmul does not head-of-line-block the PE queue)."""
                qtmp, t1, dst, sl = state
                psw = pp1.tile([128, 512], F32, tag="pp", name="psw")
                nc.tensor.matmul(psw[:], perm_t[:], qtmp[:],
                                 start=True, stop=True)
                t2 = rot_p.tile([128, 512], F32, tag="t2", name="t2")
                nc.vector.tensor_mul(t2[:], psw[:], sin_t[:, sl])
                nc.vector.tensor_add(dst[:, sl], t1[:], t2[:])

            pending = None
            for st in range(n_st):
                xts = xts_next
                if st + 1 < n_st:
                    xts_next = load_xt_strip(xt_p, st + 1)
                if st == 1:
                    for t in range(4):
                        nc.scalar.dma_start(
                            wot[t][:], wot_d.ap()[t * 128:(t + 1) * 128, :])
                sl = slice(st * 512, (st + 1) * 512)
                for eb in range(8):
                    dst = qrot[eb] if eb < 4 else krot[eb - 4]
                    ps = pp1.tile([128, 512], F32, tag="pp")
                    for db in range(n_db):
                        nc.tensor.matmul(
                            ps[:], wqk_slice(db, eb),
                            xts[:, db * 512:(db + 1) * 512],
                            start=(db == 0), stop=(db == n_db - 1))
                    qtmp = tmp_p.tile([128, 512], FR, tag="qtmp")
                    nc.scalar.copy(qtmp[:], ps[:])
                    t1 = rot_p.tile([128, 512], F32, tag="t1")
                    nc.vector.tensor_mul(t1[:], qtmp[:], cos_t[:, sl])
                    if pending is not None:
                        rope_phase2(pending)
                    pending = (qtmp, t1, dst, sl)
            rope_phase2(pending)

        if stop_after == "p1":
            with ExitStack() as ctx:
                op_ = ctx.enter_context(tc.tile_pool(name="dumo", bufs=1))
                for i in range(4):
                    nc.sync.dma_start(
                        out_d.ap()[i * 128:(i + 1) * 128, :],
                        qrot[i][:, 0:D].bitcast(mybir.dt.float32))
            nc.compile()
            return nc

        # ============ Phase P2: v projection into [v | ones] ============
        sps_p = OP(tc.tile_pool(name="sps", bufs=2, space="PSUM"))
        ops_p = OP(tc.tile_pool(name="ops", bufs=2, space="PSUM"))
        vaug_p = OP(tc.tile_pool(name="vaug", bufs=1))
        vaug = [vaug_p.tile([128, HPC * (D_K + 1)], FR, tag=f"va{i}",
                        name=f"va{i}") for i in range(n_sb)]
        with ExitStack() as ctx:
            P = ctx.enter_context
            xt_p = P(tc.tile_pool(name="xtp2", bufs=2))
            wv_p = P(tc.tile_pool(name="wv", bufs=1))
            ones_t = wv_p.tile([128, HPC], F32, name="ones_t")
            nc.gpsimd.memset(ones_t[:], 1.0)
            wv = [wv_p.tile([128, DG], FR, tag=f"wv{i}", name=f"wv{i}")
                  for i in range(n_db)]
            for db in range(n_db):
                nc.sync.dma_start(
                    wv[db][:],
                    wqkvt_d.ap()[db * 128:(db + 1) * 128, 1024:1536])
            for st in range(n_st):
                xts = load_xt_strip(xt_p, st)
                for j in range(4):
                    sb = st * 4 + j
                    ps_full = sps_p.tile([128, 1024], F32, tag="sc",
                                         name="vps")
                    ps = ps_full[:, 0:512]
                    for db in range(n_db):
                        nc.tensor.matmul(
                            ps[:],
                            xts[:, db * 512 + j * 128:db * 512 + (j + 1) * 128],
                            wv[db][:],
                            start=(db == 0), stop=(db == n_db - 1))
                    src = ps[:].rearrange("p (h c) -> p h c", c=D_K)
                    dst = vaug[sb][:].rearrange("p (h c) -> p h c", c=D_K + 1)
                    if sb % 2 == 0:
                        nc.vector.tensor_copy(dst[:, :, 0:D_K], src)
                    else:
                        nc.scalar.copy(dst[:, :, 0:D_K], src)
                    nc.vector.tensor_copy(
                        dst[:, :, D_K:D_K + 1],
                        ones_t[:].rearrange("p (h c) -> p h c", c=1))

        if stop_after == "p2":
            for i in range(4):
                nc.sync.dma_start(
                    out_d.ap()[i * 128:(i + 1) * 128, :],
                    vaug[i][:, 0:520].bitcast(mybir.dt.float32)[:, 0:512].rearrange("p n -> p n"))
            nc.compile()
            return nc

        # ============ Phase A: attention ============
        ot_p = OP(tc.tile_pool(name="ot", bufs=1))
        ot = [ot_p.tile([128, S], FR, tag=f"ot{i}", name=f"oti{i}")
              for i in range(4)]

        with ExitStack() as ctx:
            P = ctx.enter_context
            pt_p = P(tc.tile_pool(name="pt", bufs=3))
            nrm_p = P(tc.tile_pool(name="nrm", bufs=3))

            outs_p = P(tc.tile_pool(name="outs", bufs=2))
            for q2 in range(S // QT2):
                for h in range(HPC):
                    ti, po = h // 2, (h % 2) * 64
                    vlo = h * (D_K + 1)
                    q0 = q2 * QT2
                    kb_end = (q0 + QT2) // 128
                    kb_last0 = q0 // 128 + 3      # last kb writing bank 0
                    ops = ops_p.tile([D_K + 1, QT2], F32, tag="ot")
                    def emit_pv(kb, pt):
                        c0 = max(0, kb * 128 - q0)
                        b1 = max(c0, 512)
                        if c0 < 512:
                            nc.tensor.matmul(
                                ops[:, c0:512],
                                vaug[kb][:, vlo:vlo + D_K + 1],
                                pt[:, c0:512],
                                start=(kb == 0), stop=(kb == kb_last0))
                        nc.tensor.matmul(
                            ops[:, b1:QT2],
                            vaug[kb][:, vlo:vlo + D_K + 1],
                            pt[:, b1:QT2],
                            start=(kb == 0), stop=(kb == kb_end - 1))

                    pend_pv = None
                    for kb in range(kb_end):
                        c0 = max(0, kb * 128 - q0)
                        sc = sps_p.tile([128, QT2], F32, tag="sc")
                        # scores S^T (k on partitions, q on free), per bank
                        if c0 < 512:
                            nc.tensor.matmul(
                                sc[:, c0:512],
                                krot[ti][po:po + 64, kb * 128:(kb + 1) * 128],
                                qrot[ti][po:po + 64, q0 + c0:q0 + 512],
                                start=True, stop=True)
                        b1 = max(c0, 512)
                        nc.tensor.matmul(
                            sc[:, b1:QT2],
                            krot[ti][po:po + 64, kb * 128:(kb + 1) * 128],
                            qrot[ti][po:po + 64, q0 + b1:q0 + QT2],
                            start=True, stop=True)
                        pt = pt_p.tile([128, QT2], FR, tag="pt")
                        nc.scalar.activation(pt[:, c0:QT2], sc[:, c0:QT2],
                                             AF.Exp, scale=inv_sqrt_dk)
                        # causal diagonal: multiplicative post-exp (SBUF 2x)
                        if kb * 128 >= q0:
                            nc.vector.tensor_mul(pt[:, c0:c0 + 128],
                                                 pt[:, c0:c0 + 128], dmask[:])
                        if pend_pv is not None:
                            emit_pv(*pend_pv)
                        pend_pv = (kb, pt)
                    emit_pv(*pend_pv)
                    # normalize: o^T[dv, q] * (1/den[q]); the reciprocal
                    # reads PSUM partition 64 directly (cross-partition
                    # single-partition DVE read, HW-verified)
                    rinv = nrm_p.tile([1, QT2], F32, tag="rinv")
                    nc.vector.reciprocal(rinv[:], ops[D_K:D_K + 1, :])
                    den = nrm_p.tile([64, QT2], F32, tag="den")
                    nc.gpsimd.partition_broadcast(den[:], rinv[:])
                    if po == 0:
                        nc.vector.tensor_mul(ot[ti][0:64, q0:q0 + QT2],
                                             ops[0:D_K, :], den[:])
                    else:
                        onrm = nrm_p.tile([64, QT2], FR, tag="onrm")
                        nc.vector.tensor_mul(onrm[:], ops[0:D_K, :], den[:])
                        nc.sync.dma_start(ot[ti][64:128, q0:q0 + QT2],
                                          onrm[:])

            # o_proj tail (reuses the sc PSUM slots)
            if True:
                for sb in range(n_sb):
                    ssl = slice(sb * 128, (sb + 1) * 128)
                    ostage = outs_p.tile([128, D], F32, tag="ostage")
                    for eh in range(2):
                        esl = slice(eh * 512, (eh + 1) * 512)
                        ps = sps_p.tile([128, 512], F32, tag="sc")
                        for t in range(4):
                            nc.tensor.matmul(ps[:], ot[t][:, ssl],
                                             wot[t][:, esl],
                                             start=(t == 0), stop=(t == 3))
                        if eh == 0:
                            nc.vector.tensor_copy(ostage[:, esl], ps[:])
                        else:
                            nc.scalar.copy(ostage[:, esl], ps[:])
                    nc.sync.dma_start(out_d.ap()[ssl, :], ostage[:])

    nc.compile()
    return nc


def _perm128():
    """[128,128] fp32 permutation: out = P.T @ x swaps 32-row halves within
    each 64-row group. P[k, m] = 1 iff k == swap(m)."""
    p = np.zeros((128, 128), np.float32)
    for m in range(128):
        k = m + 32 if (m % 64) < 32 else m - 32
        p[k, m] = 1.0
    return p


def _rope_tables(token_positions):
    pos = np.asarray(token_positions).astype(np.float32)
    half = D_K // 2
    inv_freq = (THETA ** (-np.arange(half, dtype=np.float32) * 2.0 / D_K))
    ang = pos[None, :].astype(np.float32) * inv_freq[:, None]     # [32, S]
    cos = np.cos(ang).astype(np.float32)
    sin = np.sin(ang).astype(np.float32)
    cos128 = np.tile(cos, (4, 1))                                 # [128, S]
    sin128 = np.empty((128, pos.shape[0]), np.float32)
    for g in range(4):
        sgn = -1.0 if (g % 2 == 0) else 1.0
        sin128[g * 32:(g + 1) * 32] = sgn * sin
    return np.ascontiguousarray(cos128), np.ascontiguousarray(sin128)


def kernel(x, W_qkv, W_o, token_positions):
    out, _ = _kernel_impl(x, W_qkv, W_o, token_positions, trace=False)
    return out


def _kernel_impl(x, W_qkv, W_o, token_positions, trace=False):
    global _compiled
    from concourse.bass_utils import run_bass_kernel_spmd

    x = np.asarray(x, dtype=np.float32)
    W_qkv = np.asarray(W_qkv, dtype=np.float32)
    W_o = np.asarray(W_o, dtype=np.float32)

    if _compiled is None:
        _compiled = _build_program()
    nc = _compiled

    cos128, sin128 = _rope_tables(token_positions)
    perm = np.concatenate([np.arange(0, D_K, 2), np.arange(1, D_K, 2)])

    in_maps = []
    for c in range(N_CORES):
        b, g = divmod(c, 2)
        heads = range(g * HPC, (g + 1) * HPC)
        qrows = np.concatenate(
            [W_qkv[h * D_K:(h + 1) * D_K][perm] for h in heads])
        krows = np.concatenate(
            [W_qkv[D + h * D_K:D + (h + 1) * D_K][perm] for h in heads])
        vrows = np.concatenate(
            [W_qkv[2 * D + h * D_K:2 * D + (h + 1) * D_K] for h in heads])
        wqkvt = np.ascontiguousarray(
            np.concatenate([qrows, krows, vrows]).T)              # [1024,1536]
        wot = np.ascontiguousarray(W_o[:, g * DG:(g + 1) * DG].T)  # [512,1024]
        in_maps.append({
            "xt": np.ascontiguousarray(x[b].T),
            "wqkvt": wqkvt,
            "wot": wot,
            "perm": _perm128(),
            "cost": cos128,
            "sint": sin128,
        })

    res = run_bass_kernel_spmd(nc, in_maps, list(range(N_CORES)), trace=trace)
    out = np.empty((BS, S, D), dtype=np.float32)
    for b in range(BS):
        out[b] = res.results[2 * b]["out"] + res.results[2 * b + 1]["out"]
    return out, res.exec_time_ns



# revision 12
# speedup vs baseline: 1.0874x; 1.0874x over previous
"""Multi-head causal attention with interleaved RoPE on 8 Trainium2 cores.

nn_MultiHeadAttention: x[4,2048,1024], W_qkv[3072,1024], W_o[1024,1024],
16 heads x d_k=64, interleaved RoPE, causal softmax.

Sharding: core c = 2*b + g handles batch b (of 4) and head-group g (of 2,
8 heads each). Each core computes a full-width partial output for its batch
(o_heads @ W_o[:, group-cols]); the host sums the two partials per batch
(the "all-reduce after o_proj", done on host at gather time).

Device strategy (per core):
 - host passes x[b] transposed (xT [1024,2048]) and W slices transposed, with
   q/k rows permuted even-first so interleaved RoPE becomes rotate-half.
 - single pass over xT strips: q/k projection + RoPE AND v projection share
   each strip load (v lands in [v | ones]-augmented bf16 tiles).
 - q_rot/k_rot/v/P/o^T all held in bf16: PE stays at 1 cyc/row even for
   sub-256 moving dims, DVE gets 2x modes, SBUF fits everything resident.
 - scores computed transposed: S^T[k,q] = k_rot . q_rot per head; exp on ACT
   (1/sqrt(dk) fused into the activation scale); causal = block skipping +
   multiplicative bf16 mask on the 128x128 diagonal blocks after exp.
 - PV with lhsT = [v | ones]: softmax denominator falls out as PSUM row 64;
   normalize after PV produces o^T = exactly the lhsT o_proj needs.
 - o_proj is interleaved into the (ACT-bound) second half of attention and
   written straight from PSUM to DRAM by DMA (no staging copies).
"""

import numpy as np
from contextlib import ExitStack

NUM_HEADS = 16
D_K = 64
THETA = 10000.0
BS, S, D = 4, 2048, 1024
N_CORES = 8
HPC = NUM_HEADS // 2          # heads per core = 8
DG = HPC * D_K                # per-core head width = 512
QT2 = 1024                    # q tile (2 PSUM banks)

_compiled = None


def _build_program(stop_after=None):
    import concourse.bass as bass
    import concourse.mybir as mybir
    import concourse.tile as tile
    from concourse import bacc

    F32 = mybir.dt.float32
    FR = mybir.dt.float32r
    BF = mybir.dt.bfloat16
    AF = mybir.ActivationFunctionType

    nc = bacc.Bacc("TRN2", target_bir_lowering=False, debug=False,
                   num_devices=N_CORES)

    xt_d = nc.dram_tensor("xt", [D, S], FR, kind="ExternalInput")
    wqkvt_d = nc.dram_tensor("wqkvt", [D, 3 * DG], FR, kind="ExternalInput")
    wot_d = nc.dram_tensor("wot", [DG, D], BF, kind="ExternalInput")
    perm_d = nc.dram_tensor("perm", [128, 128], FR, kind="ExternalInput")
    cos_d = nc.dram_tensor("cost", [128, S], F32, kind="ExternalInput")
    sin_d = nc.dram_tensor("sint", [128, S], F32, kind="ExternalInput")
    out_d = nc.dram_tensor("out", [S, D], F32, kind="ExternalOutput")

    n_sb = S // 128           # 16 s-blocks
    n_st = S // 512           # 4 s-tiles
    n_db = D // 128           # 8 d-blocks
    inv_sqrt_dk = 1.0 / float(np.sqrt(D_K))

    with tile.TileContext(nc) as tc, ExitStack() as octx:
        OP = octx.enter_context
        # ---------- persistent pools (whole kernel) ----------
        qk_p = OP(tc.tile_pool(name="qk", bufs=1))
        # q_rot/k_rot: [512 e, 2048 s] as 4 tiles of [128, S] each, bf16
        qrot = [qk_p.tile([128, S], BF, tag=f"qrot{i}", name=f"qrot{i}")
                for i in range(4)]
        krot = [qk_p.tile([128, S], BF, tag=f"krot{i}", name=f"krot{i}")
                for i in range(4)]
        wot_p = OP(tc.tile_pool(name="wot", bufs=1))
        wot = [wot_p.tile([128, D], BF, tag=f"wot{i}", name=f"wott{i}")
               for i in range(4)]
        const_p = OP(tc.tile_pool(name="amisc", bufs=1))
        # multiplicative causal mask for the S^T diagonal block:
        # 1 where k <= q, 0 where k > q
        dmask = const_p.tile([128, 128], BF)
        nc.gpsimd.memset(dmask[:], 1.0)
        nc.gpsimd.affine_select(
            out=dmask[:], in_=dmask[:],
            compare_op=mybir.AluOpType.is_ge, fill=0.0, base=0,
            pattern=[[1, 128]], channel_multiplier=-1,
        )
        # v in [v | ones] augmented layout, bf16; ones columns set up front
        vaug_p = OP(tc.tile_pool(name="vaug", bufs=1))
        vaug = [vaug_p.tile([128, HPC * (D_K + 1)], BF, tag=f"va{i}",
                            name=f"va{i}") for i in range(n_sb)]
        for i in range(n_sb):
            nc.gpsimd.memset(
                vaug[i][:].rearrange("p (h c) -> p h c", c=D_K + 1)
                [:, :, D_K:D_K + 1], 1.0)

        # ============ Phase P1: q/k/v projection + RoPE, one x pass ========
        with ExitStack() as ctx:
            P = ctx.enter_context
            cs_p = P(tc.tile_pool(name="cs", bufs=1))
            xt_p = P(tc.tile_pool(name="xtp", bufs=2))
            w_p = P(tc.tile_pool(name="w", bufs=1))
            tmp_p = P(tc.tile_pool(name="tmp", bufs=5))
            rot_p = P(tc.tile_pool(name="rot", bufs=4))
            pp1 = P(tc.tile_pool(name="pp1", bufs=8, space="PSUM"))

            perm_t = cs_p.tile([128, 128], FR, name="perm_t")
            nc.scalar.dma_start(perm_t[:], perm_d.ap())
            cos_t = cs_p.tile([128, S], F32)
            sin_t = cs_p.tile([128, S], F32)
            # cos/sin land strip-by-strip so strip 0's RoPE isn't blocked
            # behind the weight loads
            def load_cs(st):
                sl = slice(st * 512, (st + 1) * 512)
                nc.scalar.dma_start(cos_t[:, sl], cos_d.ap()[:, sl])
                nc.scalar.dma_start(sin_t[:, sl], sin_d.ap()[:, sl])

            def load_xt_strip(st):
                """One batched DMA (1024 descriptors, one DGE pass): strip
                tile [128, 8*512] where chunk db holds xT[db*128:(db+1)*128,
                st*512:(st+1)*512]."""
                t = xt_p.tile([128, n_db * 512], FR, tag="xts", name="xts")
                src = xt_d.ap().rearrange("(db p) (st s) -> p db (st s)",
                                          p=128, st=n_st)
                nc.sync.dma_start(
                    t[:].rearrange("p (db s) -> p db s", db=n_db),
                    src[:, :, st * 512:(st + 1) * 512])
                return [t[:, db * 512:(db + 1) * 512] for db in range(n_db)]

            def load_w_batched(col0, col1):
                """[128, 8*(col1-col0)] tile; chunk db = wqkvt[db-block,
                col0:col1], fetched with a single rearranged DMA."""
                w = col1 - col0
                t = w_p.tile([128, n_db * w], FR, name="wbt",
                             tag=f"wbt{col0}")
                src = wqkvt_d.ap().rearrange("(db p) e -> p db e", p=128)
                nc.sync.dma_start(
                    t[:].rearrange("p (db e) -> p db e", db=n_db),
                    src[:, :, col0:col1])
                return [t[:, db * w:(db + 1) * w] for db in range(n_db)]

            # W_qk^T lo (e 0..511) as 8 per-db tiles, interleaved with 8
            # per-db strip-0 chunks so the first accumulation group streams
            # ASAP; everything later is batched (one DGE pass each)
            wqk_lo = [w_p.tile([128, DG], FR, tag=f"wqkl{i}", name=f"wqkl{i}")
                      for i in range(n_db)]
            x0 = [xt_p.tile([128, 512], FR, tag=f"x0{db}", name=f"x0t{db}",
                            bufs=1) for db in range(n_db)]
            for db in range(n_db):
                nc.sync.dma_start(
                    wqk_lo[db][:],
                    wqkvt_d.ap()[db * 128:(db + 1) * 128, 0:512])
                nc.sync.dma_start(
                    x0[db][:],
                    xt_d.ap()[db * 128:(db + 1) * 128, 0:512])
            xts_next = x0
            load_cs(0)
            wqk_hi = [load_w_batched(512, 768), load_w_batched(768, 1024)]
            wv = load_w_batched(1024, 1536)

            def wqk_slice(db, eb):
                if eb < 4:
                    return wqk_lo[db][:, eb * 128:(eb + 1) * 128]
                half = wqk_hi[(eb - 4) // 2][db]
                return half[:, ((eb - 4) % 2) * 128:((eb - 4) % 2 + 1) * 128]

            def rope_phase2(state):
                """swap-matmul + t2 + add for a previous block (lag-1 so the
                perm matmul does not head-of-line-block the PE queue)."""
                qtmp, t1, dst, sl = state
                psw = pp1.tile([128, 512], F32, tag="pp", name="psw")
                nc.tensor.matmul(psw[:], perm_t[:], qtmp[:],
                                 start=True, stop=True)
                t2 = rot_p.tile([128, 512], BF, tag="t2", name="t2")
                nc.vector.tensor_mul(t2[:], psw[:], sin_t[:, sl])
                nc.vector.tensor_add(dst[:, sl], t1[:], t2[:])

            pending = None
            for st in range(n_st):
                xts = xts_next
                if st + 1 < n_st:
                    load_cs(st + 1)
                    xts_next = load_xt_strip(st + 1)
                if st == 1:
                    for t in range(4):
                        nc.scalar.dma_start(
                            wot[t][:], wot_d.ap()[t * 128:(t + 1) * 128, :])
                sl = slice(st * 512, (st + 1) * 512)
                for eb in range(8):
                    dst = qrot[eb] if eb < 4 else krot[eb - 4]
                    ps = pp1.tile([128, 512], F32, tag="pp")
                    for db in range(n_db):
                        nc.tensor.matmul(
                            ps[:], wqk_slice(db, eb), xts[db][:],
                            start=(db == 0), stop=(db == n_db - 1))
                    qtmp = tmp_p.tile([128, 512], FR, tag="qtmp")
                    nc.scalar.copy(qtmp[:], ps[:])
                    t1 = rot_p.tile([128, 512], BF, tag="t1")
                    nc.vector.tensor_mul(t1[:], qtmp[:], cos_t[:, sl])
                    if pending is not None:
                        rope_phase2(pending)
                    pending = (qtmp, t1, dst, sl)
                # v projection for this strip's 4 s-blocks
                for j in range(4):
                    sb = st * 4 + j
                    ps = pp1.tile([128, 512], F32, tag="pp", name="vps")
                    for db in range(n_db):
                        nc.tensor.matmul(
                            ps[:], xts[db][:, j * 128:(j + 1) * 128],
                            wv[db][:],
                            start=(db == 0), stop=(db == n_db - 1))
                    src = ps[:].rearrange("p (h c) -> p h c", c=D_K)
                    dst = vaug[sb][:].rearrange("p (h c) -> p h c", c=D_K + 1)
                    nc.scalar.copy(dst[:, :, 0:D_K], src)
            rope_phase2(pending)

        if stop_after == "p1":
            with ExitStack() as ctx:
                op_ = ctx.enter_context(tc.tile_pool(name="dumo", bufs=1))
                dump = op_.tile([128, D], F32)
                for i in range(4):
                    nc.vector.tensor_copy(dump[:], qrot[i][:, 0:D])
                    nc.sync.dma_start(out_d.ap()[i * 128:(i + 1) * 128, :],
                                      dump[:])
            nc.compile()
            return nc

        # ============ Phase A: attention + interleaved o_proj ============
        # PSUM budget (16KB/partition): sc 2x4KB + ops 1x4KB + po 1x4KB.
        # ops lives only from PV start until the unnorm copy drains it to
        # SBUF, so one buffer suffices; po is the o_proj accumulator.
        sps_p = OP(tc.tile_pool(name="sps", bufs=2, space="PSUM"))
        ops_p = OP(tc.tile_pool(name="ops", bufs=1, space="PSUM"))
        po_p = OP(tc.tile_pool(name="po", bufs=1, space="PSUM"))
        ot_p = OP(tc.tile_pool(name="ot", bufs=1))
        ot = [ot_p.tile([128, S], BF, tag=f"ot{i}", name=f"oti{i}")
              for i in range(4)]

        outs_p = OP(tc.tile_pool(name="outs", bufs=4))

        def oproj_mms(po_ps, sb):
            """The 8 o_proj matmuls for s-block sb as thunks, so callers can
            spread them through the PE stream."""
            ssl = slice(sb * 128, (sb + 1) * 128)
            out = []
            for eh in range(2):
                esl = slice(eh * 512, (eh + 1) * 512)
                for t in range(4):
                    out.append(lambda eh=eh, esl=esl, t=t: nc.tensor.matmul(
                        po_ps[:, esl], ot[t][:, ssl], wot[t][:, esl],
                        start=(t == 0), stop=(t == 3)))
            return out

        def oproj_store(po_ps, sb, engine):
            ostage = outs_p.tile([128, D], F32, tag="ostage")
            if engine is nc.scalar:
                nc.scalar.copy(ostage[:], po_ps[:])
            else:
                engine.tensor_copy(ostage[:], po_ps[:])
            nc.sync.dma_start(out_d.ap()[sb * 128:(sb + 1) * 128, :],
                              ostage[:])

        with ExitStack() as ctx:
            P = ctx.enter_context
            pt_p = P(tc.tile_pool(name="pt", bufs=5))
            nrm_p = P(tc.tile_pool(name="nrm", bufs=3))

            for q2 in range(S // QT2):
                for h in range(HPC):
                    ti, po = h // 2, (h % 2) * 64
                    vlo = h * (D_K + 1)
                    q0 = q2 * QT2
                    kb_end = (q0 + QT2) // 128
                    kb_last0 = q0 // 128 + 3      # last kb writing bank 0
                    ops = ops_p.tile([D_K + 1, QT2], F32, tag="ops")
                    po_ps = None
                    po_pend = []

                    def emit_pv(kb, pt):
                        c0 = max(0, kb * 128 - q0)
                        b1 = max(c0, 512)
                        if c0 < 512:
                            nc.tensor.matmul(
                                ops[:, c0:512],
                                vaug[kb][:, vlo:vlo + D_K + 1],
                                pt[:, c0:512],
                                start=(kb == 0), stop=(kb == kb_last0))
                        nc.tensor.matmul(
                            ops[:, b1:QT2],
                            vaug[kb][:, vlo:vlo + D_K + 1],
                            pt[:, b1:QT2],
                            start=(kb == 0), stop=(kb == kb_end - 1))

                    pend_pv = []
                    for kb in range(kb_end):
                        c0 = max(0, kb * 128 - q0)
                        sc = sps_p.tile([128, QT2], F32, tag="sc")
                        # scores S^T (k on partitions, q on free), per bank
                        if c0 < 512:
                            nc.tensor.matmul(
                                sc[:, c0:512],
                                krot[ti][po:po + 64, kb * 128:(kb + 1) * 128],
                                qrot[ti][po:po + 64, q0 + c0:q0 + 512],
                                start=True, stop=True)
                        b1 = max(c0, 512)
                        nc.tensor.matmul(
                            sc[:, b1:QT2],
                            krot[ti][po:po + 64, kb * 128:(kb + 1) * 128],
                            qrot[ti][po:po + 64, q0 + b1:q0 + QT2],
                            start=True, stop=True)
                        pt = pt_p.tile([128, QT2], BF, tag="pt")
                        nc.scalar.activation(pt[:, c0:QT2], sc[:, c0:QT2],
                                             AF.Exp, scale=inv_sqrt_dk)
                        # causal diagonal: multiplicative post-exp (bf16 2x)
                        if kb * 128 >= q0:
                            nc.vector.tensor_mul(pt[:, c0:c0 + 128],
                                                 pt[:, c0:c0 + 128], dmask[:])
                        # o_proj matmuls for s-block h ride mid-head PE
                        # bubbles of this ACT-bound phase (q2==1 only);
                        # placed before the lagged PVs so QKs stay ahead
                        if q2 == 1 and 2 <= kb < 10:
                            if po_ps is None:
                                po_ps = po_p.tile([128, D], F32, tag="po",
                                                  name="po_ps")
                                po_pend = oproj_mms(po_ps, h)
                            po_pend.pop(0)()
                        # PV lags two blocks behind QK so the exp stream
                        # never waits on a score matmul
                        if len(pend_pv) >= 2:
                            emit_pv(*pend_pv.pop(0))
                        pend_pv.append((kb, pt))
                    for a in pend_pv:
                        emit_pv(*a)
                    # drain the PV accumulator to SBUF right away so the
                    # single ops buffer frees for the next head; normalize
                    # reads the SBUF copy at leisure
                    unnorm = nrm_p.tile([D_K + 1, QT2], F32, tag="unnorm")
                    nc.vector.tensor_copy(unnorm[:], ops[:])
                    if q2 == 1:
                        oproj_store(po_ps, h, nc.vector)
                    rinv = nrm_p.tile([1, QT2], F32, tag="rinv")
                    nc.vector.reciprocal(rinv[:], unnorm[D_K:D_K + 1, :])
                    den = nrm_p.tile([64, QT2], F32, tag="den")
                    nc.gpsimd.partition_broadcast(den[:], rinv[:])
                    if po == 0:
                        nc.vector.tensor_mul(ot[ti][0:64, q0:q0 + QT2],
                                             unnorm[0:D_K, :], den[:])
                    else:
                        onrm = nrm_p.tile([64, QT2], BF, tag="onrm")
                        nc.vector.tensor_mul(onrm[:], unnorm[0:D_K, :],
                                             den[:])
                        nc.sync.dma_start(ot[ti][64:128, q0:q0 + QT2],
                                          onrm[:])

            # o_proj tail for s-blocks 8..15, rotating over three PSUM slots
            # so each block's staging copy + DMA overlaps later matmuls
            pools = [sps_p, ops_p, po_p]
            tags = ["sc", "ops", "po"]
            for i, sb in enumerate(range(n_sb // 2, n_sb)):
                pool, tag = pools[i % 3], tags[i % 3]
                po_ps = pool.tile([128, D], F32, tag=tag, name="po_ps")
                for mm in oproj_mms(po_ps, sb):
                    mm()
                oproj_store(po_ps, sb, nc.scalar if i % 2 == 0 else nc.vector)

    nc.compile()
    return nc


def _perm128():
    """[128,128] fp32 permutation: out = P.T @ x swaps 32-row halves within
    each 64-row group. P[k, m] = 1 iff k == swap(m)."""
    p = np.zeros((128, 128), np.float32)
    for m in range(128):
        k = m + 32 if (m % 64) < 32 else m - 32
        p[k, m] = 1.0
    return p


def _rope_tables(token_positions):
    pos = np.asarray(token_positions).astype(np.float32)
    half = D_K // 2
    inv_freq = (THETA ** (-np.arange(half, dtype=np.float32) * 2.0 / D_K))
    ang = pos[None, :].astype(np.float32) * inv_freq[:, None]     # [32, S]
    cos = np.cos(ang).astype(np.float32)
    sin = np.sin(ang).astype(np.float32)
    cos128 = np.tile(cos, (4, 1))                                 # [128, S]
    sin128 = np.empty((128, pos.shape[0]), np.float32)
    for g in range(4):
        sgn = -1.0 if (g % 2 == 0) else 1.0
        sin128[g * 32:(g + 1) * 32] = sgn * sin
    return np.ascontiguousarray(cos128), np.ascontiguousarray(sin128)


def kernel(x, W_qkv, W_o, token_positions):
    out, _ = _kernel_impl(x, W_qkv, W_o, token_positions, trace=False)
    return out


def _kernel_impl(x, W_qkv, W_o, token_positions, trace=False):
    global _compiled
    from concourse.bass_utils import run_bass_kernel_spmd

    x = np.asarray(x, dtype=np.float32)
    W_qkv = np.asarray(W_qkv, dtype=np.float32)
    W_o = np.asarray(W_o, dtype=np.float32)

    if _compiled is None:
        _compiled = _build_program()
    nc = _compiled

    cos128, sin128 = _rope_tables(token_positions)
    perm = np.concatenate([np.arange(0, D_K, 2), np.arange(1, D_K, 2)])

    in_maps = []
    for c in range(N_CORES):
        b, g = divmod(c, 2)
        heads = range(g * HPC, (g + 1) * HPC)
        qrows = np.concatenate(
            [W_qkv[h * D_K:(h + 1) * D_K][perm] for h in heads])
        krows = np.concatenate(
            [W_qkv[D + h * D_K:D + (h + 1) * D_K][perm] for h in heads])
        vrows = np.concatenate(
            [W_qkv[2 * D + h * D_K:2 * D + (h + 1) * D_K] for h in heads])
        wqkvt = np.ascontiguousarray(
            np.concatenate([qrows, krows, vrows]).T)              # [1024,1536]
        import ml_dtypes
        wot = np.ascontiguousarray(
            W_o[:, g * DG:(g + 1) * DG].T.astype(ml_dtypes.bfloat16))
        in_maps.append({
            "xt": np.ascontiguousarray(x[b].T),
            "wqkvt": wqkvt,
            "wot": wot,
            "perm": _perm128(),
            "cost": cos128,
            "sint": sin128,
        })

    res = run_bass_kernel_spmd(nc, in_maps, list(range(N_CORES)), trace=trace)
    out = np.empty((BS, S, D), dtype=np.float32)
    for b in range(BS):
        out[b] = res.results[2 * b]["out"] + res.results[2 * b + 1]["out"]
    return out, res.exec_time_ns


# revision 19
# speedup vs baseline: 1.2034x; 1.1067x over previous
"""Multi-head causal attention with interleaved RoPE on 8 Trainium2 cores.

nn_MultiHeadAttention: x[4,2048,1024], W_qkv[3072,1024], W_o[1024,1024],
16 heads x d_k=64, interleaved RoPE, causal softmax.

Sharding: core c = 2*b + g handles batch b (of 4) and head-group g (of 2,
8 heads each). Each core computes a full-width partial output for its batch
(o_heads @ W_o[:, group-cols]); the host sums the two partials per batch
(the "all-reduce after o_proj", done on host at gather time).

Device schedule (per core), engineered against the TimelineSim cost model:
 - everything matmul-adjacent is bf16 (x, W, q_rot/k_rot, v, P, o^T): PE is
   1 cyc/row at any moving size, DMA bytes halve, and all of it stays in
   SBUF across the kernel.
 - phase 1: x strips 0,1 -> q/k projection + RoPE (rotate-half via a
   host-permuted W + perm matmul) and v projection, one pass per strip.
   Strip 0 runs db-major with 8 open PSUM groups so PE streams while the
   prologue DMAs land.
 - overlap phase: strips 2,3 are interleaved with the ENTIRE first half of
   attention (q <= 1024 only needs k rows < 1024 = strips 0,1, by
   causality). Attention here uses 512-wide q-tiles so its PSUM footprint
   (3 score slots + 2 accumulators) coexists with a 3-slot projection ring.
 - second half of attention (q2=1) is ACT(exp)-bound: QK matmuls run two
   blocks ahead of the lagged PV matmuls, o_proj for the first 8 s-blocks
   rides the PE bubbles, staged through SBUF and DMA'd out.
 - o_proj tail rotates over all four PSUM slots so PE never waits.
"""

import numpy as np
from contextlib import ExitStack

NUM_HEADS = 16
D_K = 64
THETA = 10000.0
BS, S, D = 4, 2048, 1024
N_CORES = 8
HPC = NUM_HEADS // 2          # heads per core = 8
DG = HPC * D_K                # per-core head width = 512

_compiled = None


def _build_program():
    import concourse.bass as bass
    import concourse.mybir as mybir
    import concourse.tile as tile
    from concourse import bacc

    F32 = mybir.dt.float32
    FR = mybir.dt.float32r
    BF = mybir.dt.bfloat16
    AF = mybir.ActivationFunctionType

    nc = bacc.Bacc("TRN2", target_bir_lowering=False, debug=False,
                   num_devices=N_CORES)

    xt_d = nc.dram_tensor("xt", [D, S], BF, kind="ExternalInput")
    wqkvt_d = nc.dram_tensor("wqkvt", [D, 3 * DG], BF, kind="ExternalInput")
    wot_d = nc.dram_tensor("wot", [DG, D], BF, kind="ExternalInput")
    perm_d = nc.dram_tensor("perm", [128, 128], BF, kind="ExternalInput")
    cos_d = nc.dram_tensor("cost", [128, S], BF, kind="ExternalInput")
    sin_d = nc.dram_tensor("sint", [128, S], BF, kind="ExternalInput")
    out_d = nc.dram_tensor("out", [S, D], F32, kind="ExternalOutput")

    n_sb = S // 128           # 16 s-blocks
    n_st = S // 512           # 4 s-tiles
    n_db = D // 128           # 8 d-blocks
    inv_sqrt_dk = 1.0 / float(np.sqrt(D_K))

    with tile.TileContext(nc) as tc, ExitStack() as octx:
        OP = octx.enter_context
        # ---------- persistent pools (whole kernel) ----------
        qk_p = OP(tc.tile_pool(name="qk", bufs=1))
        qrot = [qk_p.tile([128, S], BF, tag=f"qrot{i}", name=f"qrot{i}")
                for i in range(4)]
        krot = [qk_p.tile([128, S], BF, tag=f"krot{i}", name=f"krot{i}")
                for i in range(4)]
        wot_p = OP(tc.tile_pool(name="wot", bufs=1))
        wot = [wot_p.tile([128, D], BF, tag=f"wot{i}", name=f"wott{i}")
               for i in range(4)]
        ot_p = OP(tc.tile_pool(name="ot", bufs=1))
        ot = [ot_p.tile([128, S], BF, tag=f"ot{i}", name=f"oti{i}")
              for i in range(4)]
        const_p = OP(tc.tile_pool(name="amisc", bufs=1))
        # multiplicative causal mask for the S^T diagonal block:
        # 1 where k <= q, 0 where k > q
        dmask = const_p.tile([128, 128], BF)
        nc.gpsimd.memset(dmask[:], 1.0)
        nc.gpsimd.affine_select(
            out=dmask[:], in_=dmask[:],
            compare_op=mybir.AluOpType.is_ge, fill=0.0, base=0,
            pattern=[[1, 128]], channel_multiplier=-1,
        )
        # v in [v | ones] augmented layout, bf16; ones columns set up front
        vaug_p = OP(tc.tile_pool(name="vaug", bufs=1))
        vaug = [vaug_p.tile([128, HPC * (D_K + 1)], BF, tag=f"va{i}",
                            name=f"va{i}") for i in range(n_sb)]
        for i in range(n_sb):
            nc.gpsimd.memset(
                vaug[i][:].rearrange("p (h c) -> p h c", c=D_K + 1)
                [:, :, D_K:D_K + 1], 1.0)
        pt_p = OP(tc.tile_pool(name="pt", bufs=6))
        nrm_p = OP(tc.tile_pool(name="nrm", bufs=3))

        # ---------------- attention building blocks ----------------
        def qk_exp_mask(sc_pool, qt, ti, po, q0, kb):
            """QK matmuls + exp + diag mask for one (head, k-block) against
            q-range [q0, q0+qt); returns the bf16 probability tile."""
            c0 = max(0, kb * 128 - q0)
            sc = sc_pool.tile([128, qt], F32, tag="sc", name="sc")
            lo = c0
            while lo < qt:                      # per-512 PSUM bank chunks
                hi = min(lo - lo % 512 + 512, qt)
                nc.tensor.matmul(
                    sc[:, lo:hi],
                    krot[ti][po:po + 64, kb * 128:(kb + 1) * 128],
                    qrot[ti][po:po + 64, q0 + lo:q0 + hi],
                    start=True, stop=True)
                lo = hi
            pt = pt_p.tile([128, qt], BF, tag="pt", name="pt")
            nc.scalar.activation(pt[:, c0:qt], sc[:, c0:qt],
                                 AF.Exp, scale=inv_sqrt_dk)
            if kb * 128 >= q0:                  # causal diagonal, bf16 2x
                nc.vector.tensor_mul(pt[:, c0:c0 + 128],
                                     pt[:, c0:c0 + 128], dmask[:])
            return pt

        def emit_pv(ops, qt, h, q0, kb_end, kb, pt):
            vlo = h * (D_K + 1)
            c0 = max(0, kb * 128 - q0)
            lo = c0
            while lo < qt:
                hi = min(lo - lo % 512 + 512, qt)
                last = kb_end - 1 if hi == qt else (q0 + hi) // 128 - 1
                nc.tensor.matmul(
                    ops[:, lo:hi],
                    vaug[kb][:, vlo:vlo + D_K + 1],
                    pt[:, lo:hi],
                    start=(kb == 0), stop=(kb == last))
                lo = hi

        def normalize(ops, qt, ti, po, q0):
            """Drain the PV accumulator to SBUF right away (frees the PSUM
            slot), then recip/broadcast/scale into o^T."""
            unnorm = nrm_p.tile([D_K + 1, qt], F32, tag="unnorm",
                                name="unnorm")
            nc.vector.tensor_copy(unnorm[:], ops[:])
            rinv = nrm_p.tile([1, qt], F32, tag="rinv", name="rinv")
            nc.vector.reciprocal(rinv[:], unnorm[D_K:D_K + 1, :])
            den = nrm_p.tile([64, qt], F32, tag="den", name="den")
            nc.gpsimd.partition_broadcast(den[:], rinv[:])
            if po == 0:
                nc.vector.tensor_mul(ot[ti][0:64, q0:q0 + qt],
                                     unnorm[0:D_K, :], den[:])
            else:
                onrm = nrm_p.tile([64, qt], BF, tag="onrm", name="onrm")
                nc.vector.tensor_mul(onrm[:], unnorm[0:D_K, :], den[:])
                nc.sync.dma_start(ot[ti][64:128, q0:q0 + qt], onrm[:])

        # ============ projection + first-half attention ============
        with ExitStack() as p1s:
            P1 = p1s.enter_context
            cs_p = P1(tc.tile_pool(name="cs", bufs=1))
            xt_p = P1(tc.tile_pool(name="xtp", bufs=2))
            w_p = P1(tc.tile_pool(name="w", bufs=1))
            tmp_p = P1(tc.tile_pool(name="tmp", bufs=5))
            rot_p = P1(tc.tile_pool(name="rot", bufs=4))

            perm_t = cs_p.tile([128, 128], BF, name="perm_t")
            nc.scalar.dma_start(perm_t[:], perm_d.ap())
            cos_t = cs_p.tile([128, S], BF)
            sin_t = cs_p.tile([128, S], BF)

            def load_cs(st):
                sl = slice(st * 512, (st + 1) * 512)
                nc.scalar.dma_start(cos_t[:, sl], cos_d.ap()[:, sl])
                nc.scalar.dma_start(sin_t[:, sl], sin_d.ap()[:, sl])

            def load_xt_strip(st):
                """One batched DMA (1024 descriptors, one DGE pass)."""
                t = xt_p.tile([128, n_db * 512], BF, tag="xts", name="xts")
                src = xt_d.ap().rearrange("(db p) (st s) -> p db (st s)",
                                          p=128, st=n_st)
                nc.sync.dma_start(
                    t[:].rearrange("p (db s) -> p db s", db=n_db),
                    src[:, :, st * 512:(st + 1) * 512])
                return [t[:, db * 512:(db + 1) * 512] for db in range(n_db)]

            # W_qk^T as 8 per-db full-width tiles interleaved with the 8
            # per-db strip-0 chunks: as chunk db lands, all 8 e-blocks can
            # consume it (db-major sweep below)
            wqkf = [w_p.tile([128, 2 * DG], BF, tag=f"wqkf{i}",
                             name=f"wqkf{i}") for i in range(n_db)]
            wv_t = w_p.tile([128, n_db * DG], BF, name="wv_t")

            def wqk_slice(db, eb):
                return wqkf[db][:, eb * 128:(eb + 1) * 128]

            wv = [wv_t[:, db * DG:(db + 1) * DG] for db in range(n_db)]

            rope_pend = []

            def rope_phase2():
                """swap-matmul + t2 + add for a previous block (lag-1 so the
                perm matmul doesn't head-of-line-block the PE queue). The
                final add runs on GPSIMD: DVE is loaded during the overlap
                phase, Pool is idle."""
                pp, qtmp, t1, dst, sl = rope_pend.pop(0)
                psw = pp.tile([128, 512], F32, tag="pp", name="psw")
                nc.tensor.matmul(psw[:], perm_t[:], qtmp[:],
                                 start=True, stop=True)
                t2 = rot_p.tile([128, 512], BF, tag="t2", name="t2")
                nc.vector.tensor_mul(t2[:], psw[:], sin_t[:, sl])
                nc.gpsimd.tensor_add(dst[:, sl], t1[:], t2[:])

            def rope_tail(pp, eb, ps, sl):
                qtmp = tmp_p.tile([128, 512], BF, tag="qtmp")
                nc.scalar.copy(qtmp[:], ps[:])
                t1 = rot_p.tile([128, 512], BF, tag="t1")
                nc.vector.tensor_mul(t1[:], qtmp[:], cos_t[:, sl])
                if rope_pend:
                    rope_phase2()
                dst = qrot[eb] if eb < 4 else krot[eb - 4]
                rope_pend.append((pp, qtmp, t1, dst, sl))

            def emit_eb(pp, st, xts, eb):
                sl = slice(st * 512, (st + 1) * 512)
                ps = pp.tile([128, 512], F32, tag="pp", name="ps")
                for db in range(n_db):
                    nc.tensor.matmul(
                        ps[:], wqk_slice(db, eb), xts[db][:],
                        start=(db == 0), stop=(db == n_db - 1))
                rope_tail(pp, eb, ps, sl)

            def emit_v(pp, st, xts, j):
                sb = st * 4 + j
                ps = pp.tile([128, 512], F32, tag="pp", name="vps")
                for db in range(n_db):
                    nc.tensor.matmul(
                        ps[:], xts[db][:, j * 128:(j + 1) * 128], wv[db][:],
                        start=(db == 0), stop=(db == n_db - 1))
                src = ps[:].rearrange("p (h c) -> p h c", c=D_K)
                dst = vaug[sb][:].rearrange("p (h c) -> p h c", c=D_K + 1)
                nc.scalar.copy(dst[:, :, 0:D_K], src)

            # ---- strips 0,1: deep PSUM ring, db-major strip 0 ----
            with ExitStack() as s01:
                pp8 = s01.enter_context(
                    tc.tile_pool(name="pp8", bufs=8, space="PSUM"))
                x0_p = s01.enter_context(tc.tile_pool(name="x0p", bufs=1))
                x0 = [x0_p.tile([128, 512], BF, tag=f"x0{db}",
                                name=f"x0t{db}") for db in range(n_db)]
                for db in range(n_db):
                    nc.sync.dma_start(
                        wqkf[db][:],
                        wqkvt_d.ap()[db * 128:(db + 1) * 128, 0:1024])
                    nc.sync.dma_start(
                        x0[db][:],
                        xt_d.ap()[db * 128:(db + 1) * 128, 0:512])
                load_cs(0)
                nc.sync.dma_start(
                    wv_t[:].rearrange("p (db e) -> p db e", db=n_db),
                    wqkvt_d.ap().rearrange("(db p) e -> p db e", p=128)
                    [:, :, 1024:1536])
                load_cs(1)
                xts1 = load_xt_strip(1)
                for t in range(4):
                    nc.scalar.dma_start(
                        wot[t][:], wot_d.ap()[t * 128:(t + 1) * 128, :])
                # strip 0, db-major: 8 open accumulation groups
                pss = [pp8.tile([128, 512], F32, tag="pp", name="pss")
                       for _ in range(8)]
                for db in range(n_db):
                    for eb in range(8):
                        nc.tensor.matmul(
                            pss[eb][:], wqk_slice(db, eb), x0[db][:],
                            start=(db == 0), stop=(db == n_db - 1))
                for eb in range(8):
                    rope_tail(pp8, eb, pss[eb], slice(0, 512))
                for j in range(4):
                    emit_v(pp8, 0, x0, j)
                # strip 1, eb-major
                load_cs(2)
                xts2 = load_xt_strip(2)
                for eb in range(8):
                    emit_eb(pp8, 1, xts1, eb)
                for j in range(4):
                    emit_v(pp8, 1, xts1, j)
                while rope_pend:
                    rope_phase2()

            # ---- overlap: strips 2,3 interleaved with all of q2=0 ----
            # (q < 1024 attends only to k < 1024 = strips 0,1)
            with ExitStack() as ovl:
                sc0_p = ovl.enter_context(
                    tc.tile_pool(name="sc0", bufs=3, space="PSUM"))
                ops0_p = ovl.enter_context(
                    tc.tile_pool(name="ops0", bufs=2, space="PSUM"))
                pp3 = ovl.enter_context(
                    tc.tile_pool(name="pp3", bufs=3, space="PSUM"))

                load_cs(3)
                xts3 = load_xt_strip(3)
                strip_units = (
                    [lambda eb=eb: emit_eb(pp3, 2, xts2, eb)
                     for eb in range(8)] +
                    [lambda j=j: emit_v(pp3, 2, xts2, j) for j in range(4)] +
                    [lambda eb=eb: emit_eb(pp3, 3, xts3, eb)
                     for eb in range(8)] +
                    [lambda j=j: emit_v(pp3, 3, xts3, j) for j in range(4)])
                su_i = 0
                step = 0

                QT = 512
                for ti in range(4):
                    for qt_i in range(2):
                        q0 = qt_i * 512
                        kb_end = (q0 + QT) // 128
                        ops2 = [ops0_p.tile([D_K + 1, QT], F32, tag="ops0",
                                            name="ops0")
                                for _ in range(2)]
                        pend_pv = []
                        for kb in range(kb_end):
                            for s in range(2):
                                pt = qk_exp_mask(sc0_p, QT, ti, s * 64,
                                                 q0, kb)
                                if len(pend_pv) >= 3:
                                    emit_pv(*pend_pv.pop(0))
                                pend_pv.append(
                                    (ops2[s], QT, 2 * ti + s, q0, kb_end,
                                     kb, pt))
                            step += 1
                            if step % 2 == 0 and su_i < len(strip_units):
                                strip_units[su_i]()
                                su_i += 1
                        for a in pend_pv:
                            emit_pv(*a)
                        for s in range(2):
                            normalize(ops2[s], QT, ti, s * 64, q0)
                while su_i < len(strip_units):
                    strip_units[su_i]()
                    su_i += 1
                while rope_pend:
                    rope_phase2()

        # ============ second-half attention + o_proj ============
        QT2 = 1024
        sps_p = OP(tc.tile_pool(name="sps", bufs=2, space="PSUM"))
        ops_p = OP(tc.tile_pool(name="ops", bufs=1, space="PSUM"))
        po_p = OP(tc.tile_pool(name="po", bufs=1, space="PSUM"))
        outs_p = OP(tc.tile_pool(name="outs", bufs=4))

        def oproj_mms(po_ps, sb):
            ssl = slice(sb * 128, (sb + 1) * 128)
            out = []
            for eh in range(2):
                esl = slice(eh * 512, (eh + 1) * 512)
                for t in range(4):
                    out.append(lambda eh=eh, esl=esl, t=t: nc.tensor.matmul(
                        po_ps[:, esl], ot[t][:, ssl], wot[t][:, esl],
                        start=(t == 0), stop=(t == 3)))
            return out

        def oproj_store(po_ps, sb, engine):
            ostage = outs_p.tile([128, D], F32, tag="ostage", name="ostage")
            if engine is nc.scalar:
                nc.scalar.copy(ostage[:], po_ps[:])
            else:
                engine.tensor_copy(ostage[:], po_ps[:])
            nc.sync.dma_start(out_d.ap()[sb * 128:(sb + 1) * 128, :],
                              ostage[:])

        for h in range(HPC):
            ti, po = h // 2, (h % 2) * 64
            ops = ops_p.tile([D_K + 1, QT2], F32, tag="ops", name="ops")
            po_ps = None
            po_pend = []
            pend_pv = []
            for kb in range(16):
                pt = qk_exp_mask(sps_p, QT2, ti, po, QT2, kb)
                # o_proj matmuls placed before the lagged PVs so the QK
                # stream stays ahead of the exp stream
                if 2 <= kb < 10:
                    if po_ps is None:
                        po_ps = po_p.tile([128, D], F32, tag="po",
                                          name="po_ps")
                        po_pend = oproj_mms(po_ps, h)
                    po_pend.pop(0)()
                if len(pend_pv) >= 2:
                    emit_pv(*pend_pv.pop(0))
                pend_pv.append((ops, QT2, h, QT2, 16, kb, pt))
            for a in pend_pv:
                emit_pv(*a)
            oproj_store(po_ps, h, nc.vector)
            normalize(ops, QT2, ti, po, QT2)

        # o_proj tail for s-blocks 8..15, rotating over all four PSUM
        # slots so no matmul ever waits on a staging copy
        pools = [sps_p, sps_p, ops_p, po_p]
        tags = ["sc", "sc", "ops", "po"]
        for i, sb in enumerate(range(n_sb // 2, n_sb)):
            pool, tag = pools[i % 4], tags[i % 4]
            po_ps = pool.tile([128, D], F32, tag=tag, name="po_ps")
            for mm in oproj_mms(po_ps, sb):
                mm()
            oproj_store(po_ps, sb, nc.scalar if i % 2 == 0 else nc.vector)

    nc.compile()
    return nc


def _perm128():
    """[128,128] permutation: out = P.T @ x swaps 32-row halves within
    each 64-row group. P[k, m] = 1 iff k == swap(m)."""
    p = np.zeros((128, 128), np.float32)
    for m in range(128):
        k = m + 32 if (m % 64) < 32 else m - 32
        p[k, m] = 1.0
    return p


def _rope_tables(token_positions):
    pos = np.asarray(token_positions).astype(np.float32)
    half = D_K // 2
    inv_freq = (THETA ** (-np.arange(half, dtype=np.float32) * 2.0 / D_K))
    ang = pos[None, :].astype(np.float32) * inv_freq[:, None]     # [32, S]
    cos = np.cos(ang).astype(np.float32)
    sin = np.sin(ang).astype(np.float32)
    cos128 = np.tile(cos, (4, 1))                                 # [128, S]
    sin128 = np.empty((128, pos.shape[0]), np.float32)
    for g in range(4):
        sgn = -1.0 if (g % 2 == 0) else 1.0
        sin128[g * 32:(g + 1) * 32] = sgn * sin
    return np.ascontiguousarray(cos128), np.ascontiguousarray(sin128)


def kernel(x, W_qkv, W_o, token_positions):
    out, _ = _kernel_impl(x, W_qkv, W_o, token_positions, trace=False)
    return out


def _kernel_impl(x, W_qkv, W_o, token_positions, trace=False):
    global _compiled
    import ml_dtypes
    from concourse.bass_utils import run_bass_kernel_spmd

    BF = ml_dtypes.bfloat16
    x = np.asarray(x, dtype=np.float32)
    W_qkv = np.asarray(W_qkv, dtype=np.float32)
    W_o = np.asarray(W_o, dtype=np.float32)

    if _compiled is None:
        _compiled = _build_program()
    nc = _compiled

    cos128, sin128 = _rope_tables(token_positions)
    perm = np.concatenate([np.arange(0, D_K, 2), np.arange(1, D_K, 2)])

    in_maps = []
    for c in range(N_CORES):
        b, g = divmod(c, 2)
        heads = range(g * HPC, (g + 1) * HPC)
        qrows = np.concatenate(
            [W_qkv[h * D_K:(h + 1) * D_K][perm] for h in heads])
        krows = np.concatenate(
            [W_qkv[D + h * D_K:D + (h + 1) * D_K][perm] for h in heads])
        vrows = np.concatenate(
            [W_qkv[2 * D + h * D_K:2 * D + (h + 1) * D_K] for h in heads])
        wqkvt = np.ascontiguousarray(
            np.concatenate([qrows, krows, vrows]).T.astype(BF))  # [1024,1536]
        wotm = np.ascontiguousarray(
            W_o[:, g * DG:(g + 1) * DG].T.astype(BF))            # [512,1024]
        in_maps.append({
            "xt": np.ascontiguousarray(x[b].T.astype(BF)),
            "wqkvt": wqkvt,
            "wot": wotm,
            "perm": _perm128().astype(BF),
            "cost": cos128.astype(BF),
            "sint": sin128.astype(BF),
        })

    res = run_bass_kernel_spmd(nc, in_maps, list(range(N_CORES)), trace=trace)
    out = np.empty((BS, S, D), dtype=np.float32)
    for b in range(BS):
        out[b] = res.results[2 * b]["out"] + res.results[2 * b + 1]["out"]
    return out, res.exec_time_ns


# revision 21
# speedup vs baseline: 1.2621x; 1.0487x over previous
"""Multi-head causal attention with interleaved RoPE on 8 Trainium2 cores.

nn_MultiHeadAttention: x[4,2048,1024], W_qkv[3072,1024], W_o[1024,1024],
16 heads x d_k=64, interleaved RoPE, causal softmax.

Sharding: core c = 2*b + g handles batch b (of 4) and head-group g (of 2,
8 heads each). Each core computes a full-width partial output for its batch
(o_heads @ W_o[:, group-cols]); the host sums the two partials per batch
(the "all-reduce after o_proj", done on host at gather time).

Device schedule (per core), engineered against the TimelineSim cost model:
 - everything matmul-adjacent is bf16 (x, W, q_rot/k_rot, v, P, o^T): PE is
   1 cyc/row at any moving size, DMA bytes halve, and all of it stays in
   SBUF across the kernel.
 - phase 1: x strips 0,1 -> q/k projection + RoPE (rotate-half via a
   host-permuted W + perm matmul) and v projection, one pass per strip.
   Strip 0 runs db-major with 8 open PSUM groups so PE streams while the
   prologue DMAs land.
 - overlap phase: strips 2,3 are interleaved with the ENTIRE first half of
   attention (q <= 1024 only needs k rows < 1024 = strips 0,1, by
   causality). Attention here uses 512-wide q-tiles so its PSUM footprint
   (3 score slots + 2 accumulators) coexists with a 3-slot projection ring.
 - second half of attention (q2=1) is ACT(exp)-bound: QK matmuls run two
   blocks ahead of the lagged PV matmuls, o_proj for the first 8 s-blocks
   rides the PE bubbles, staged through SBUF and DMA'd out.
 - o_proj tail rotates over all four PSUM slots so PE never waits.
"""

import numpy as np
from contextlib import ExitStack

NUM_HEADS = 16
D_K = 64
THETA = 10000.0
BS, S, D = 4, 2048, 1024
N_CORES = 8
HPC = NUM_HEADS // 2          # heads per core = 8
DG = HPC * D_K                # per-core head width = 512

_compiled = None


def _build_program():
    import concourse.bass as bass
    import concourse.mybir as mybir
    import concourse.tile as tile
    from concourse import bacc

    F32 = mybir.dt.float32
    FR = mybir.dt.float32r
    BF = mybir.dt.bfloat16
    AF = mybir.ActivationFunctionType

    nc = bacc.Bacc("TRN2", target_bir_lowering=False, debug=False,
                   num_devices=N_CORES)

    xt_d = nc.dram_tensor("xt", [D, S], BF, kind="ExternalInput")
    wqkvt_d = nc.dram_tensor("wqkvt", [D, 3 * DG], BF, kind="ExternalInput")
    wot_d = nc.dram_tensor("wot", [DG, D], BF, kind="ExternalInput")
    perm_d = nc.dram_tensor("perm", [128, 128], BF, kind="ExternalInput")
    cos_d = nc.dram_tensor("cost", [128, S], BF, kind="ExternalInput")
    sin_d = nc.dram_tensor("sint", [128, S], BF, kind="ExternalInput")
    out_d = nc.dram_tensor("out", [S, D], F32, kind="ExternalOutput")

    n_sb = S // 128           # 16 s-blocks
    n_st = S // 512           # 4 s-tiles
    n_db = D // 128           # 8 d-blocks
    inv_sqrt_dk = 1.0 / float(np.sqrt(D_K))

    with tile.TileContext(nc) as tc, ExitStack() as octx:
        OP = octx.enter_context
        # ---------- persistent pools (whole kernel) ----------
        qk_p = OP(tc.tile_pool(name="qk", bufs=1))
        qrot = [qk_p.tile([128, S], BF, tag=f"qrot{i}", name=f"qrot{i}")
                for i in range(4)]
        krot = [qk_p.tile([128, S], BF, tag=f"krot{i}", name=f"krot{i}")
                for i in range(4)]
        wot_p = OP(tc.tile_pool(name="wot", bufs=1))
        wot = [wot_p.tile([128, D], BF, tag=f"wot{i}", name=f"wott{i}")
               for i in range(4)]
        ot_p = OP(tc.tile_pool(name="ot", bufs=1))
        ot = [ot_p.tile([128, S], BF, tag=f"ot{i}", name=f"oti{i}")
              for i in range(4)]
        const_p = OP(tc.tile_pool(name="amisc", bufs=1))
        # multiplicative causal mask for the S^T diagonal block:
        # 1 where k <= q, 0 where k > q
        dmask = const_p.tile([128, 128], BF)
        nc.gpsimd.memset(dmask[:], 1.0)
        nc.gpsimd.affine_select(
            out=dmask[:], in_=dmask[:],
            compare_op=mybir.AluOpType.is_ge, fill=0.0, base=0,
            pattern=[[1, 128]], channel_multiplier=-1,
        )
        # v in [v | ones] augmented layout, bf16; ones columns set up front
        vaug_p = OP(tc.tile_pool(name="vaug", bufs=1))
        vaug = [vaug_p.tile([128, HPC * (D_K + 1)], BF, tag=f"va{i}",
                            name=f"va{i}") for i in range(n_sb)]
        for i in range(n_sb):
            nc.gpsimd.memset(
                vaug[i][:].rearrange("p (h c) -> p h c", c=D_K + 1)
                [:, :, D_K:D_K + 1], 1.0)
        pt_p = OP(tc.tile_pool(name="pt", bufs=6))
        nrm_p = OP(tc.tile_pool(name="nrm", bufs=3))

        # ---------------- attention building blocks ----------------
        def qk_exp_mask(sc_pool, qt, ti, po, q0, kb):
            """QK matmuls + exp + diag mask for one (head, k-block) against
            q-range [q0, q0+qt); returns the bf16 probability tile."""
            c0 = max(0, kb * 128 - q0)
            sc = sc_pool.tile([128, qt], F32, tag="sc", name="sc")
            lo = c0
            while lo < qt:                      # per-512 PSUM bank chunks
                hi = min(lo - lo % 512 + 512, qt)
                nc.tensor.matmul(
                    sc[:, lo:hi],
                    krot[ti][po:po + 64, kb * 128:(kb + 1) * 128],
                    qrot[ti][po:po + 64, q0 + lo:q0 + hi],
                    start=True, stop=True)
                lo = hi
            pt = pt_p.tile([128, qt], BF, tag="pt", name="pt")
            nc.scalar.activation(pt[:, c0:qt], sc[:, c0:qt],
                                 AF.Exp, scale=inv_sqrt_dk)
            if kb * 128 >= q0:                  # causal diagonal, bf16 2x
                nc.vector.tensor_mul(pt[:, c0:c0 + 128],
                                     pt[:, c0:c0 + 128], dmask[:])
            return pt

        def emit_pv(ops, qt, h, q0, kb_end, kb, pt):
            vlo = h * (D_K + 1)
            c0 = max(0, kb * 128 - q0)
            lo = c0
            while lo < qt:
                hi = min(lo - lo % 512 + 512, qt)
                last = kb_end - 1 if hi == qt else (q0 + hi) // 128 - 1
                nc.tensor.matmul(
                    ops[:, lo:hi],
                    vaug[kb][:, vlo:vlo + D_K + 1],
                    pt[:, lo:hi],
                    start=(kb == 0), stop=(kb == last))
                lo = hi

        def normalize(ops, qt, ti, po, q0):
            """Drain the PV accumulator to SBUF right away (frees the PSUM
            slot), then recip/broadcast/scale into o^T."""
            unnorm = nrm_p.tile([D_K + 1, qt], F32, tag="unnorm",
                                name="unnorm")
            nc.vector.tensor_copy(unnorm[:], ops[:])
            rinv = nrm_p.tile([1, qt], F32, tag="rinv", name="rinv")
            nc.vector.reciprocal(rinv[:], unnorm[D_K:D_K + 1, :])
            den = nrm_p.tile([64, qt], F32, tag="den", name="den")
            nc.gpsimd.partition_broadcast(den[:], rinv[:])
            if po == 0:
                nc.vector.tensor_mul(ot[ti][0:64, q0:q0 + qt],
                                     unnorm[0:D_K, :], den[:])
            else:
                onrm = nrm_p.tile([64, qt], BF, tag="onrm", name="onrm")
                nc.vector.tensor_mul(onrm[:], unnorm[0:D_K, :], den[:])
                nc.sync.dma_start(ot[ti][64:128, q0:q0 + qt], onrm[:])

        # ============ projection + first-half attention ============
        with ExitStack() as p1s:
            P1 = p1s.enter_context
            cs_p = P1(tc.tile_pool(name="cs", bufs=1))
            xt_p = P1(tc.tile_pool(name="xtp", bufs=2))
            w_p = P1(tc.tile_pool(name="w", bufs=1))
            tmp_p = P1(tc.tile_pool(name="tmp", bufs=5))
            rot_p = P1(tc.tile_pool(name="rot", bufs=4))

            perm_t = cs_p.tile([128, 128], BF, name="perm_t")
            nc.scalar.dma_start(perm_t[:], perm_d.ap())
            cos_t = cs_p.tile([128, S], BF)
            sin_t = cs_p.tile([128, S], BF)

            def load_cs(st):
                sl = slice(st * 512, (st + 1) * 512)
                nc.scalar.dma_start(cos_t[:, sl], cos_d.ap()[:, sl])
                nc.scalar.dma_start(sin_t[:, sl], sin_d.ap()[:, sl])

            def load_xt_strip(st):
                """One batched DMA (1024 descriptors, one DGE pass)."""
                t = xt_p.tile([128, n_db * 512], BF, tag="xts", name="xts")
                src = xt_d.ap().rearrange("(db p) (st s) -> p db (st s)",
                                          p=128, st=n_st)
                nc.sync.dma_start(
                    t[:].rearrange("p (db s) -> p db s", db=n_db),
                    src[:, :, st * 512:(st + 1) * 512])
                return [t[:, db * 512:(db + 1) * 512] for db in range(n_db)]

            # W_qk^T as 8 per-db full-width tiles interleaved with the 8
            # per-db strip-0 chunks: as chunk db lands, all 8 e-blocks can
            # consume it (db-major sweep below)
            wqkf = [w_p.tile([128, 2 * DG], BF, tag=f"wqkf{i}",
                             name=f"wqkf{i}") for i in range(n_db)]
            wv_t = w_p.tile([128, n_db * DG], BF, name="wv_t")

            def wqk_slice(db, eb):
                return wqkf[db][:, eb * 128:(eb + 1) * 128]

            wv = [wv_t[:, db * DG:(db + 1) * DG] for db in range(n_db)]

            rope_pend = []

            def rope_phase2():
                """swap-matmul + t2 + add for a previous block (lag-1 so the
                perm matmul doesn't head-of-line-block the PE queue). The
                final add runs on GPSIMD: DVE is loaded during the overlap
                phase, Pool is idle."""
                pp, qtmp, t1, dst, sl = rope_pend.pop(0)
                psw = pp.tile([128, 512], F32, tag="pp", name="psw")
                nc.tensor.matmul(psw[:], perm_t[:], qtmp[:],
                                 start=True, stop=True)
                t2 = rot_p.tile([128, 512], BF, tag="t2", name="t2")
                nc.vector.tensor_mul(t2[:], psw[:], sin_t[:, sl])
                nc.gpsimd.tensor_add(dst[:, sl], t1[:], t2[:])

            def rope_tail(pp, eb, ps, sl):
                qtmp = tmp_p.tile([128, 512], BF, tag="qtmp")
                nc.scalar.copy(qtmp[:], ps[:])
                t1 = rot_p.tile([128, 512], BF, tag="t1")
                nc.vector.tensor_mul(t1[:], qtmp[:], cos_t[:, sl])
                if rope_pend:
                    rope_phase2()
                dst = qrot[eb] if eb < 4 else krot[eb - 4]
                rope_pend.append((pp, qtmp, t1, dst, sl))

            def emit_eb(pp, st, xts, eb):
                sl = slice(st * 512, (st + 1) * 512)
                ps = pp.tile([128, 512], F32, tag="pp", name="ps")
                for db in range(n_db):
                    nc.tensor.matmul(
                        ps[:], wqk_slice(db, eb), xts[db][:],
                        start=(db == 0), stop=(db == n_db - 1))
                rope_tail(pp, eb, ps, sl)

            def emit_v(pp, st, xts, j):
                sb = st * 4 + j
                ps = pp.tile([128, 512], F32, tag="pp", name="vps")
                for db in range(n_db):
                    nc.tensor.matmul(
                        ps[:], xts[db][:, j * 128:(j + 1) * 128], wv[db][:],
                        start=(db == 0), stop=(db == n_db - 1))
                src = ps[:].rearrange("p (h c) -> p h c", c=D_K)
                dst = vaug[sb][:].rearrange("p (h c) -> p h c", c=D_K + 1)
                nc.scalar.copy(dst[:, :, 0:D_K], src)

            # ---- strips 0,1: deep PSUM ring, db-major strip 0 ----
            with ExitStack() as s01:
                pp8 = s01.enter_context(
                    tc.tile_pool(name="pp8", bufs=8, space="PSUM"))
                x0_p = s01.enter_context(tc.tile_pool(name="x0p", bufs=1))
                x0 = [x0_p.tile([128, 512], BF, tag=f"x0{db}",
                                name=f"x0t{db}") for db in range(n_db)]
                for db in range(n_db):
                    nc.sync.dma_start(
                        wqkf[db][:],
                        wqkvt_d.ap()[db * 128:(db + 1) * 128, 0:1024])
                    nc.sync.dma_start(
                        x0[db][:],
                        xt_d.ap()[db * 128:(db + 1) * 128, 0:512])
                load_cs(0)
                nc.sync.dma_start(
                    wv_t[:].rearrange("p (db e) -> p db e", db=n_db),
                    wqkvt_d.ap().rearrange("(db p) e -> p db e", p=128)
                    [:, :, 1024:1536])
                load_cs(1)
                xts1 = load_xt_strip(1)
                for t in range(4):
                    nc.scalar.dma_start(
                        wot[t][:], wot_d.ap()[t * 128:(t + 1) * 128, :])
                # strip 0, db-major: 8 open accumulation groups
                pss = [pp8.tile([128, 512], F32, tag="pp", name="pss")
                       for _ in range(8)]
                for db in range(n_db):
                    for eb in range(8):
                        nc.tensor.matmul(
                            pss[eb][:], wqk_slice(db, eb), x0[db][:],
                            start=(db == 0), stop=(db == n_db - 1))
                for eb in range(8):
                    rope_tail(pp8, eb, pss[eb], slice(0, 512))
                for j in range(4):
                    emit_v(pp8, 0, x0, j)
                # strip 1, eb-major
                load_cs(2)
                xts2 = load_xt_strip(2)
                for eb in range(8):
                    emit_eb(pp8, 1, xts1, eb)
                for j in range(4):
                    emit_v(pp8, 1, xts1, j)
                while rope_pend:
                    rope_phase2()

            # ---- overlap: strips 2,3 interleaved with all of q2=0 ----
            # (q < 1024 attends only to k < 1024 = strips 0,1)
            with ExitStack() as ovl:
                sc0_p = ovl.enter_context(
                    tc.tile_pool(name="sc0", bufs=3, space="PSUM"))
                ops0_p = ovl.enter_context(
                    tc.tile_pool(name="ops0", bufs=2, space="PSUM"))
                pp3 = ovl.enter_context(
                    tc.tile_pool(name="pp3", bufs=3, space="PSUM"))

                load_cs(3)
                xts3 = load_xt_strip(3)
                strip_units = (
                    [lambda eb=eb: emit_eb(pp3, 2, xts2, eb)
                     for eb in range(8)] +
                    [lambda j=j: emit_v(pp3, 2, xts2, j) for j in range(4)] +
                    [lambda eb=eb: emit_eb(pp3, 3, xts3, eb)
                     for eb in range(8)] +
                    [lambda j=j: emit_v(pp3, 3, xts3, j) for j in range(4)])
                su_i = 0
                step = 0

                QT = 512
                for ti in range(4):
                    for qt_i in range(2):
                        q0 = qt_i * 512
                        kb_end = (q0 + QT) // 128
                        ops2 = [ops0_p.tile([D_K + 1, QT], F32, tag="ops0",
                                            name="ops0")
                                for _ in range(2)]
                        pend_pv = []
                        for kb in range(kb_end):
                            for s in range(2):
                                pt = qk_exp_mask(sc0_p, QT, ti, s * 64,
                                                 q0, kb)
                                if len(pend_pv) >= 3:
                                    emit_pv(*pend_pv.pop(0))
                                pend_pv.append(
                                    (ops2[s], QT, 2 * ti + s, q0, kb_end,
                                     kb, pt))
                            step += 1
                            if step % 2 == 0 and su_i < len(strip_units):
                                strip_units[su_i]()
                                su_i += 1
                        for a in pend_pv:
                            emit_pv(*a)
                        for s in range(2):
                            normalize(ops2[s], QT, ti, s * 64, q0)
                while su_i < len(strip_units):
                    strip_units[su_i]()
                    su_i += 1
                while rope_pend:
                    rope_phase2()

        # ============ second-half attention + o_proj ============
        QT2 = 1024
        sps_p = OP(tc.tile_pool(name="sps", bufs=2, space="PSUM"))
        ops_p = OP(tc.tile_pool(name="ops", bufs=1, space="PSUM"))
        po_p = OP(tc.tile_pool(name="po", bufs=1, space="PSUM"))
        outs_p = OP(tc.tile_pool(name="outs", bufs=4))

        def oproj_mms(po_ps, sb, t_order=(0, 1, 2, 3)):
            """o_proj matmuls t-major so callers can defer the tiles whose
            ot columns land last."""
            ssl = slice(sb * 128, (sb + 1) * 128)
            out = []
            for t in t_order:
                for eh in range(2):
                    esl = slice(eh * 512, (eh + 1) * 512)
                    out.append(lambda esl=esl, t=t: nc.tensor.matmul(
                        po_ps[:, esl], ot[t][:, ssl], wot[t][:, esl],
                        start=(t == t_order[0]), stop=(t == t_order[-1])))
            return out

        def oproj_store(po_ps, sb, engine):
            ostage = outs_p.tile([128, D], F32, tag="ostage", name="ostage")
            if engine is nc.scalar:
                nc.scalar.copy(ostage[:], po_ps[:])
            else:
                engine.tensor_copy(ostage[:], po_ps[:])
            nc.sync.dma_start(out_d.ap()[sb * 128:(sb + 1) * 128, :],
                              ostage[:])

        # kb visit order alternates full-height blocks (1024-wide exps) with
        # diagonal blocks (short exps) so ACT always has a long exp in
        # flight to hide the short ones' dependency latency
        kb_order = list(range(16))
        # per 512-column PSUM chunk, the first/last contributing kb in
        # emission order (start/stop accumulation flags)
        contrib = {0: [kb for kb in kb_order if max(0, kb * 128 - QT2) < 512],
                   512: kb_order[:]}
        pv_first = {lo: ks[0] for lo, ks in contrib.items()}
        pv_last = {lo: ks[-1] for lo, ks in contrib.items()}

        def emit_pv_q21(ops, h, kb, pt):
            vlo = h * (D_K + 1)
            c0 = max(0, kb * 128 - QT2)
            for lo in (0, 512):
                if c0 >= lo + 512:
                    continue
                nc.tensor.matmul(
                    ops[:, max(c0, lo):lo + 512],
                    vaug[kb][:, vlo:vlo + D_K + 1],
                    pt[:, max(c0, lo):lo + 512],
                    start=(kb == pv_first[lo]), stop=(kb == pv_last[lo]))

        # within each ti, the po=64 head (whose o^T lands via DMA) runs
        # first so the final ot write before the tail is the fast DVE path
        h_order = [1, 0, 3, 2, 5, 4, 7, 6]
        for hi_i, h in enumerate(h_order):
            ti, po = h // 2, (h % 2) * 64
            ops = ops_p.tile([D_K + 1, QT2], F32, tag="ops", name="ops")
            po_ps = None
            po_pend = []
            pend_pv = []
            for u, kb in enumerate(kb_order):
                pt = qk_exp_mask(sps_p, QT2, ti, po, QT2, kb)
                # o_proj matmuls placed before the lagged PVs so the QK
                # stream stays ahead of the exp stream
                if 2 <= u < 10:
                    if po_ps is None:
                        po_ps = po_p.tile([128, D], F32, tag="po",
                                          name="po_ps")
                        po_pend = oproj_mms(po_ps, hi_i)
                    po_pend.pop(0)()
                if len(pend_pv) >= 2:
                    emit_pv_q21(*pend_pv.pop(0))
                pend_pv.append((ops, h, kb, pt))
            for a in pend_pv:
                emit_pv_q21(*a)
            oproj_store(po_ps, hi_i, nc.vector)
            normalize(ops, QT2, ti, po, QT2)

        # o_proj tail for s-blocks 8..15: two-phase per block — the six
        # matmuls reading ot[0..2] run immediately (those columns are long
        # written), the two reading ot[3] (written by the final heads) are
        # deferred; four PSUM slots stay rotating so PE never idles
        pools = [sps_p, sps_p, ops_p, po_p]
        tags = ["sc", "sc", "ops", "po"]
        pend_stores = []

        def flush_tail():
            po_ps, sb, late, i = pend_stores.pop(0)
            for mm in late:
                mm()
            oproj_store(po_ps, sb, nc.scalar if i % 2 == 0 else nc.vector)

        for i, sb in enumerate(range(n_sb // 2, n_sb)):
            pool, tag = pools[i % 4], tags[i % 4]
            po_ps = pool.tile([128, D], F32, tag=tag, name="po_ps")
            mms = oproj_mms(po_ps, sb, t_order=(0, 1, 2, 3))
            for mm in mms[:6]:
                mm()
            pend_stores.append((po_ps, sb, mms[6:], i))
            if len(pend_stores) >= 3:
                flush_tail()
        while pend_stores:
            flush_tail()

    nc.compile()
    return nc


def _perm128():
    """[128,128] permutation: out = P.T @ x swaps 32-row halves within
    each 64-row group. P[k, m] = 1 iff k == swap(m)."""
    p = np.zeros((128, 128), np.float32)
    for m in range(128):
        k = m + 32 if (m % 64) < 32 else m - 32
        p[k, m] = 1.0
    return p


def _rope_tables(token_positions):
    pos = np.asarray(token_positions).astype(np.float32)
    half = D_K // 2
    inv_freq = (THETA ** (-np.arange(half, dtype=np.float32) * 2.0 / D_K))
    ang = pos[None, :].astype(np.float32) * inv_freq[:, None]     # [32, S]
    cos = np.cos(ang).astype(np.float32)
    sin = np.sin(ang).astype(np.float32)
    cos128 = np.tile(cos, (4, 1))                                 # [128, S]
    sin128 = np.empty((128, pos.shape[0]), np.float32)
    for g in range(4):
        sgn = -1.0 if (g % 2 == 0) else 1.0
        sin128[g * 32:(g + 1) * 32] = sgn * sin
    return np.ascontiguousarray(cos128), np.ascontiguousarray(sin128)


def kernel(x, W_qkv, W_o, token_positions):
    out, _ = _kernel_impl(x, W_qkv, W_o, token_positions, trace=False)
    return out


def _kernel_impl(x, W_qkv, W_o, token_positions, trace=False):
    global _compiled
    import ml_dtypes
    from concourse.bass_utils import run_bass_kernel_spmd

    BF = ml_dtypes.bfloat16
    x = np.asarray(x, dtype=np.float32)
    W_qkv = np.asarray(W_qkv, dtype=np.float32)
    W_o = np.asarray(W_o, dtype=np.float32)

    if _compiled is None:
        _compiled = _build_program()
    nc = _compiled

    cos128, sin128 = _rope_tables(token_positions)
    perm = np.concatenate([np.arange(0, D_K, 2), np.arange(1, D_K, 2)])

    in_maps = []
    for c in range(N_CORES):
        b, g = divmod(c, 2)
        heads = range(g * HPC, (g + 1) * HPC)
        qrows = np.concatenate(
            [W_qkv[h * D_K:(h + 1) * D_K][perm] for h in heads])
        krows = np.concatenate(
            [W_qkv[D + h * D_K:D + (h + 1) * D_K][perm] for h in heads])
        vrows = np.concatenate(
            [W_qkv[2 * D + h * D_K:2 * D + (h + 1) * D_K] for h in heads])
        wqkvt = np.ascontiguousarray(
            np.concatenate([qrows, krows, vrows]).T.astype(BF))  # [1024,1536]
        wotm = np.ascontiguousarray(
            W_o[:, g * DG:(g + 1) * DG].T.astype(BF))            # [512,1024]
        in_maps.append({
            "xt": np.ascontiguousarray(x[b].T.astype(BF)),
            "wqkvt": wqkvt,
            "wot": wotm,
            "perm": _perm128().astype(BF),
            "cost": cos128.astype(BF),
            "sint": sin128.astype(BF),
        })

    res = run_bass_kernel_spmd(nc, in_maps, list(range(N_CORES)), trace=trace)
    out = np.empty((BS, S, D), dtype=np.float32)
    for b in range(BS):
        out[b] = res.results[2 * b]["out"] + res.results[2 * b + 1]["out"]
    return out, res.exec_time_ns


# revision 33
# speedup vs baseline: 1.2835x; 1.0170x over previous
"""Multi-head causal attention with interleaved RoPE on 8 Trainium2 cores.

nn_MultiHeadAttention: x[4,2048,1024], W_qkv[3072,1024], W_o[1024,1024],
16 heads x d_k=64, interleaved RoPE, causal softmax.

Sharding: core c = 2*b + g handles batch b (of 4) and head-group g (of 2,
8 heads each). Each core computes a full-width partial output for its batch
(o_heads @ W_o[:, group-cols]); the host sums the two partials per batch
(the "all-reduce after o_proj", done on host at gather time).

Device schedule (per core), engineered against the TimelineSim cost model:
 - everything matmul-adjacent is bf16 (x, W, q_rot/k_rot, v, P, o^T): PE is
   1 cyc/row at any moving size, DMA bytes halve, and all of it stays in
   SBUF across the kernel.
 - phase 1: x strips 0,1 -> q/k projection + RoPE (rotate-half via a
   host-permuted W + perm matmul) and v projection, one pass per strip.
   Strip 0 runs db-major with 8 open PSUM groups so PE streams while the
   prologue DMAs land.
 - overlap phase: strips 2,3 are interleaved with the ENTIRE first half of
   attention (q <= 1024 only needs k rows < 1024 = strips 0,1, by
   causality). Attention here uses 512-wide q-tiles so its PSUM footprint
   (3 score slots + 2 accumulators) coexists with a 3-slot projection ring.
 - second half of attention (q2=1) is ACT(exp)-bound: QK matmuls run two
   blocks ahead of the lagged PV matmuls, o_proj for the first 8 s-blocks
   rides the PE bubbles, staged through SBUF and DMA'd out.
 - o_proj tail rotates over all four PSUM slots so PE never waits.
"""

import numpy as np
from contextlib import ExitStack

NUM_HEADS = 16
D_K = 64
THETA = 10000.0
BS, S, D = 4, 2048, 1024
N_CORES = 8
HPC = NUM_HEADS // 2          # heads per core = 8
DG = HPC * D_K                # per-core head width = 512

_compiled = None


def _build_program():
    import concourse.bass as bass
    import concourse.mybir as mybir
    import concourse.tile as tile
    from concourse import bacc

    F32 = mybir.dt.float32
    FR = mybir.dt.float32r
    BF = mybir.dt.bfloat16
    AF = mybir.ActivationFunctionType

    nc = bacc.Bacc("TRN2", target_bir_lowering=False, debug=False,
                   num_devices=N_CORES)

    xt_d = nc.dram_tensor("xt", [D, S], BF, kind="ExternalInput")
    wqkvt_d = nc.dram_tensor("wqkvt", [D, 3 * DG], BF, kind="ExternalInput")
    wot_d = nc.dram_tensor("wot", [DG, D], BF, kind="ExternalInput")
    perm_d = nc.dram_tensor("perm", [128, 128], BF, kind="ExternalInput")
    cos_d = nc.dram_tensor("cost", [128, S], BF, kind="ExternalInput")
    sin_d = nc.dram_tensor("sint", [128, S], BF, kind="ExternalInput")
    out_d = nc.dram_tensor("out", [S, D], BF, kind="ExternalOutput")

    n_sb = S // 128           # 16 s-blocks
    n_st = S // 512           # 4 s-tiles
    n_db = D // 128           # 8 d-blocks
    inv_sqrt_dk = 1.0 / float(np.sqrt(D_K))

    with tile.TileContext(nc) as tc, ExitStack() as octx:
        OP = octx.enter_context
        # ---------- persistent pools (whole kernel) ----------
        qk_p = OP(tc.tile_pool(name="qk", bufs=1))
        qrot = [qk_p.tile([128, S], BF, tag=f"qrot{i}", name=f"qrot{i}")
                for i in range(4)]
        krot = [qk_p.tile([128, S], BF, tag=f"krot{i}", name=f"krot{i}")
                for i in range(4)]
        wot_p = OP(tc.tile_pool(name="wot", bufs=1))
        wot = [wot_p.tile([128, D], BF, tag=f"wot{i}", name=f"wott{i}")
               for i in range(4)]
        ot_p = OP(tc.tile_pool(name="ot", bufs=1))
        ot = [ot_p.tile([128, S], BF, tag=f"ot{i}", name=f"oti{i}")
              for i in range(4)]
        const_p = OP(tc.tile_pool(name="amisc", bufs=1))
        # multiplicative causal mask for the S^T diagonal block:
        # 1 where k <= q, 0 where k > q
        dmask = const_p.tile([128, 128], BF)
        nc.gpsimd.memset(dmask[:], 1.0)
        nc.gpsimd.affine_select(
            out=dmask[:], in_=dmask[:],
            compare_op=mybir.AluOpType.is_ge, fill=0.0, base=0,
            pattern=[[1, 128]], channel_multiplier=-1,
        )
        # v in [v | ones] augmented layout, bf16; ones columns set up front
        vaug_p = OP(tc.tile_pool(name="vaug", bufs=1))
        vaug = [vaug_p.tile([128, HPC * (D_K + 1)], BF, tag=f"va{i}",
                            name=f"va{i}") for i in range(n_sb)]
        for i in range(n_sb):
            nc.gpsimd.memset(
                vaug[i][:].rearrange("p (h c) -> p h c", c=D_K + 1)
                [:, :, D_K:D_K + 1], 1.0)
        pt_p = OP(tc.tile_pool(name="pt", bufs=6))
        nrm_p = OP(tc.tile_pool(name="nrm", bufs=3))

        # ---------------- attention building blocks ----------------
        def qk_exp_mask(sc_pool, qt, ti, po, q0, kb):
            """QK matmuls + exp + diag mask for one (head, k-block) against
            q-range [q0, q0+qt); returns the bf16 probability tile."""
            c0 = max(0, kb * 128 - q0)
            sc = sc_pool.tile([128, qt], F32, tag="sc", name="sc")
            lo = c0
            while lo < qt:                      # per-512 PSUM bank chunks
                hi = min(lo - lo % 512 + 512, qt)
                nc.tensor.matmul(
                    sc[:, lo:hi],
                    krot[ti][po:po + 64, kb * 128:(kb + 1) * 128],
                    qrot[ti][po:po + 64, q0 + lo:q0 + hi],
                    start=True, stop=True)
                lo = hi
            pt = pt_p.tile([128, qt], BF, tag="pt", name="pt")
            nc.scalar.activation(pt[:, c0:qt], sc[:, c0:qt],
                                 AF.Exp, scale=inv_sqrt_dk)
            if kb * 128 >= q0:                  # causal diagonal, bf16 2x
                nc.vector.tensor_mul(pt[:, c0:c0 + 128],
                                     pt[:, c0:c0 + 128], dmask[:])
            return pt

        def emit_pv(ops, qt, h, q0, kb_end, kb, pt):
            vlo = h * (D_K + 1)
            c0 = max(0, kb * 128 - q0)
            lo = c0
            while lo < qt:
                hi = min(lo - lo % 512 + 512, qt)
                last = kb_end - 1 if hi == qt else (q0 + hi) // 128 - 1
                nc.tensor.matmul(
                    ops[:, lo:hi],
                    vaug[kb][:, vlo:vlo + D_K + 1],
                    pt[:, lo:hi],
                    start=(kb == 0), stop=(kb == last))
                lo = hi

        def normalize(ops, qt, ti, po, q0, lo=0, hi=None):
            """Drain the PV accumulator to SBUF right away (frees the PSUM
            slot), then recip/broadcast/scale into o^T. lo/hi select a
            column sub-range (used to pipeline the last head's epilogue)."""
            hi = qt if hi is None else hi
            w = hi - lo
            unnorm = nrm_p.tile([D_K + 1, qt], F32, tag="unnorm",
                                name="unnorm")
            nc.vector.tensor_copy(unnorm[:, 0:w], ops[:, lo:hi])
            rinv = nrm_p.tile([1, qt], F32, tag="rinv", name="rinv")
            nc.vector.reciprocal(rinv[:, 0:w], unnorm[D_K:D_K + 1, 0:w])
            den = nrm_p.tile([64, qt], F32, tag="den", name="den")
            nc.gpsimd.partition_broadcast(den[:, 0:w], rinv[:, 0:w])
            if po == 0:
                nc.vector.tensor_mul(ot[ti][0:64, q0 + lo:q0 + hi],
                                     unnorm[0:D_K, 0:w], den[:, 0:w])
            else:
                onrm = nrm_p.tile([64, qt], BF, tag="onrm", name="onrm")
                nc.vector.tensor_mul(onrm[:, 0:w], unnorm[0:D_K, 0:w],
                                     den[:, 0:w])
                nc.sync.dma_start(ot[ti][64:128, q0 + lo:q0 + hi],
                                  onrm[:, 0:w])

        # ============ projection + first-half attention ============
        with ExitStack() as p1s:
            P1 = p1s.enter_context
            cs_p = P1(tc.tile_pool(name="cs", bufs=1))
            xt_p = P1(tc.tile_pool(name="xtp", bufs=2))
            w_p = P1(tc.tile_pool(name="w", bufs=1))
            tmp_p = P1(tc.tile_pool(name="tmp", bufs=5))
            rot_p = P1(tc.tile_pool(name="rot", bufs=4))

            perm_t = cs_p.tile([128, 128], BF, name="perm_t")
            nc.scalar.dma_start(perm_t[:], perm_d.ap())
            cos_t = cs_p.tile([128, S], BF)
            sin_t = cs_p.tile([128, S], BF)

            def load_cs(st):
                sl = slice(st * 512, (st + 1) * 512)
                nc.scalar.dma_start(cos_t[:, sl], cos_d.ap()[:, sl])
                nc.scalar.dma_start(sin_t[:, sl], sin_d.ap()[:, sl])

            def load_xt_strip(st):
                """One batched DMA (1024 descriptors, one DGE pass)."""
                t = xt_p.tile([128, n_db * 512], BF, tag="xts", name="xts")
                src = xt_d.ap().rearrange("(db p) (st s) -> p db (st s)",
                                          p=128, st=n_st)
                nc.sync.dma_start(
                    t[:].rearrange("p (db s) -> p db s", db=n_db),
                    src[:, :, st * 512:(st + 1) * 512])
                return [t[:, db * 512:(db + 1) * 512] for db in range(n_db)]

            # W_qk^T as 8 per-db full-width tiles interleaved with the 8
            # per-db strip-0 chunks: as chunk db lands, all 8 e-blocks can
            # consume it (db-major sweep below)
            wqkf = [w_p.tile([128, 2 * DG], BF, tag=f"wqkf{i}",
                             name=f"wqkf{i}") for i in range(n_db)]
            wv_t = w_p.tile([128, n_db * DG], BF, name="wv_t")

            def wqk_slice(db, eb):
                return wqkf[db][:, eb * 128:(eb + 1) * 128]

            wv = [wv_t[:, db * DG:(db + 1) * DG] for db in range(n_db)]

            rope_pend = []

            def rope_phase2():
                """swap-matmul + t2 + add for a previous block (lag-1 so the
                perm matmul doesn't head-of-line-block the PE queue). The
                final add runs on GPSIMD: DVE is loaded during the overlap
                phase, Pool is idle."""
                pp, qtmp, t1, dst, sl = rope_pend.pop(0)
                psw = pp.tile([128, 512], F32, tag="pp", name="psw")
                nc.tensor.matmul(psw[:], perm_t[:], qtmp[:],
                                 start=True, stop=True)
                t2 = rot_p.tile([128, 512], BF, tag="t2", name="t2")
                nc.vector.tensor_mul(t2[:], psw[:], sin_t[:, sl])
                nc.gpsimd.tensor_add(dst[:, sl], t1[:], t2[:])

            def rope_tail(pp, eb, ps, sl):
                qtmp = tmp_p.tile([128, 512], BF, tag="qtmp")
                nc.scalar.copy(qtmp[:], ps[:])
                t1 = rot_p.tile([128, 512], BF, tag="t1")
                nc.vector.tensor_mul(t1[:], qtmp[:], cos_t[:, sl])
                if rope_pend:
                    rope_phase2()
                dst = qrot[eb] if eb < 4 else krot[eb - 4]
                rope_pend.append((pp, qtmp, t1, dst, sl))

            def emit_eb(pp, st, xts, eb):
                sl = slice(st * 512, (st + 1) * 512)
                ps = pp.tile([128, 512], F32, tag="pp", name="ps")
                for db in range(n_db):
                    nc.tensor.matmul(
                        ps[:], wqk_slice(db, eb), xts[db][:],
                        start=(db == 0), stop=(db == n_db - 1))
                rope_tail(pp, eb, ps, sl)

            def emit_v(pp, st, xts, j):
                sb = st * 4 + j
                ps = pp.tile([128, 512], F32, tag="pp", name="vps")
                for db in range(n_db):
                    nc.tensor.matmul(
                        ps[:], xts[db][:, j * 128:(j + 1) * 128], wv[db][:],
                        start=(db == 0), stop=(db == n_db - 1))
                src = ps[:].rearrange("p (h c) -> p h c", c=D_K)
                dst = vaug[sb][:].rearrange("p (h c) -> p h c", c=D_K + 1)
                nc.scalar.copy(dst[:, :, 0:D_K], src)

            # ---- strips 0,1: deep PSUM ring, db-major strip 0 ----
            with ExitStack() as s01:
                pp8 = s01.enter_context(
                    tc.tile_pool(name="pp8", bufs=8, space="PSUM"))
                x0_p = s01.enter_context(tc.tile_pool(name="x0p", bufs=1))
                x0 = [x0_p.tile([128, 512], BF, tag=f"x0{db}",
                                name=f"x0t{db}") for db in range(n_db)]
                for db in range(n_db):
                    nc.sync.dma_start(
                        wqkf[db][:],
                        wqkvt_d.ap()[db * 128:(db + 1) * 128, 0:1024])
                    nc.sync.dma_start(
                        x0[db][:],
                        xt_d.ap()[db * 128:(db + 1) * 128, 0:512])
                load_cs(0)
                load_cs(1)
                xts1 = load_xt_strip(1)
                nc.sync.dma_start(
                    wv_t[:].rearrange("p (db e) -> p db e", db=n_db),
                    wqkvt_d.ap().rearrange("(db p) e -> p db e", p=128)
                    [:, :, 1024:1536])
                for t in range(4):
                    nc.scalar.dma_start(
                        wot[t][:], wot_d.ap()[t * 128:(t + 1) * 128, :])
                # strip 0, db-major: 8 open accumulation groups
                pss = [pp8.tile([128, 512], F32, tag="pp", name="pss")
                       for _ in range(8)]
                for db in range(n_db):
                    for eb in range(8):
                        nc.tensor.matmul(
                            pss[eb][:], wqk_slice(db, eb), x0[db][:],
                            start=(db == 0), stop=(db == n_db - 1))
                for eb in range(8):
                    rope_tail(pp8, eb, pss[eb], slice(0, 512))
                # strip 1 e-blocks before strip 0's v so PE doesn't wait on
                # the wv load; v projections follow once wv is resident
                load_cs(2)
                xts2 = load_xt_strip(2)
                for eb in range(8):
                    emit_eb(pp8, 1, xts1, eb)
                for j in range(4):
                    emit_v(pp8, 0, x0, j)
                for j in range(4):
                    emit_v(pp8, 1, xts1, j)
                while rope_pend:
                    rope_phase2()

            # ---- overlap: strips 2,3 interleaved with all of q2=0 ----
            # (q < 1024 attends only to k < 1024 = strips 0,1)
            with ExitStack() as ovl:
                sc0_p = ovl.enter_context(
                    tc.tile_pool(name="sc0", bufs=3, space="PSUM"))
                ops0_p = ovl.enter_context(
                    tc.tile_pool(name="ops0", bufs=2, space="PSUM"))
                pp3 = ovl.enter_context(
                    tc.tile_pool(name="pp3", bufs=3, space="PSUM"))

                load_cs(3)
                xts3 = load_xt_strip(3)
                strip_units = (
                    [lambda eb=eb: emit_eb(pp3, 2, xts2, eb)
                     for eb in range(8)] +
                    [lambda j=j: emit_v(pp3, 2, xts2, j) for j in range(4)] +
                    [lambda eb=eb: emit_eb(pp3, 3, xts3, eb)
                     for eb in range(8)] +
                    [lambda j=j: emit_v(pp3, 3, xts3, j) for j in range(4)])
                su_i = 0
                step = 0

                QT = 512
                for ti in range(4):
                    for qt_i in range(2):
                        q0 = qt_i * 512
                        kb_end = (q0 + QT) // 128
                        ops2 = [ops0_p.tile([D_K + 1, QT], F32, tag="ops0",
                                            name="ops0")
                                for _ in range(2)]
                        pend_pv = []
                        for kb in range(kb_end):
                            for s in range(2):
                                pt = qk_exp_mask(sc0_p, QT, ti, s * 64,
                                                 q0, kb)
                                if len(pend_pv) >= 3:
                                    emit_pv(*pend_pv.pop(0))
                                pend_pv.append(
                                    (ops2[s], QT, 2 * ti + s, q0, kb_end,
                                     kb, pt))
                            step += 1
                            if step % 2 == 0 and su_i < len(strip_units):
                                strip_units[su_i]()
                                su_i += 1
                        for a in pend_pv:
                            emit_pv(*a)
                        for s in range(2):
                            normalize(ops2[s], QT, ti, s * 64, q0)
                while su_i < len(strip_units):
                    strip_units[su_i]()
                    su_i += 1
                while rope_pend:
                    rope_phase2()

        # ============ second-half attention + o_proj ============
        QT2 = 1024
        sps_p = OP(tc.tile_pool(name="sps", bufs=2, space="PSUM"))
        ops_p = OP(tc.tile_pool(name="ops", bufs=1, space="PSUM"))
        po_p = OP(tc.tile_pool(name="po", bufs=1, space="PSUM"))
        outs_p = OP(tc.tile_pool(name="outs", bufs=4))

        def oproj_mms(po_ps, sb, t_order=(0, 1, 2, 3)):
            """o_proj matmuls t-major so callers can defer the tiles whose
            ot columns land last."""
            ssl = slice(sb * 128, (sb + 1) * 128)
            out = []
            for t in t_order:
                for eh in range(2):
                    esl = slice(eh * 512, (eh + 1) * 512)
                    out.append(lambda esl=esl, t=t: nc.tensor.matmul(
                        po_ps[:, esl], ot[t][:, ssl], wot[t][:, esl],
                        start=(t == t_order[0]), stop=(t == t_order[-1])))
            return out

        def oproj_store(po_ps, sb, engine):
            ostage = outs_p.tile([128, D], BF, tag="ostage", name="ostage")
            if engine is nc.scalar:
                nc.scalar.copy(ostage[:], po_ps[:])
            else:
                engine.tensor_copy(ostage[:], po_ps[:])
            nc.sync.dma_start(out_d.ap()[sb * 128:(sb + 1) * 128, :],
                              ostage[:])

        # kb visit order alternates full-height blocks (1024-wide exps) with
        # diagonal blocks (short exps) so ACT always has a long exp in
        # flight to hide the short ones' dependency latency
        kb_order = list(range(16))
        # per 512-column PSUM chunk, the first/last contributing kb in
        # emission order (start/stop accumulation flags)
        contrib = {0: [kb for kb in kb_order if max(0, kb * 128 - QT2) < 512],
                   512: kb_order[:]}
        pv_first = {lo: ks[0] for lo, ks in contrib.items()}
        pv_last = {lo: ks[-1] for lo, ks in contrib.items()}

        def emit_pv_q21(ops, h, kb, pt):
            vlo = h * (D_K + 1)
            c0 = max(0, kb * 128 - QT2)
            for lo in (0, 512):
                if c0 >= lo + 512:
                    continue
                nc.tensor.matmul(
                    ops[:, max(c0, lo):lo + 512],
                    vaug[kb][:, vlo:vlo + D_K + 1],
                    pt[:, max(c0, lo):lo + 512],
                    start=(kb == pv_first[lo]), stop=(kb == pv_last[lo]))

        # within each ti, the po=64 head (whose o^T lands via DMA) runs
        # first so the final ot write before the tail is the fast DVE path.
        # The two pending PVs carry ACROSS head boundaries: the next head's
        # first QKs are emitted before the previous head's last PVs, so the
        # exp stream never sees a boundary bubble.
        h_order = [1, 0, 3, 2, 5, 4, 7, 6]
        pend_pv = []
        pend_fin = []                 # (ops, hi_i, po_ps, ti, po) to close

        def pop_pv():
            ops, h, kb, pt = pend_pv.pop(0)
            emit_pv_q21(ops, h, kb, pt)
            if h == h_order[-1] and kb == pv_last[0]:
                # last head: normalize the first half as soon as its PSUM
                # chunk closes, so the tail's deferred matmuls unblock early
                normalize(ops, QT2, h // 2, (h % 2) * 64, QT2, 0, 512)
            if kb == kb_order[-1] and pend_fin:
                ops_f, hi_f, po_ps_f, ti_f, po_f = pend_fin.pop(0)
                oproj_store(po_ps_f, hi_f, nc.vector)
                if hi_f == 7:
                    normalize(ops_f, QT2, ti_f, po_f, QT2, 512, QT2)
                else:
                    normalize(ops_f, QT2, ti_f, po_f, QT2)

        for hi_i, h in enumerate(h_order):
            ti, po = h // 2, (h % 2) * 64
            ops = ops_p.tile([D_K + 1, QT2], F32, tag="ops", name="ops")
            po_ps = None
            po_pend = []
            for u, kb in enumerate(kb_order):
                pt = qk_exp_mask(sps_p, QT2, ti, po, QT2, kb)
                # o_proj matmuls placed before the lagged PVs so the QK
                # stream stays ahead of the exp stream
                if 4 <= u < 12:
                    if po_ps is None:
                        po_ps = po_p.tile([128, D], F32, tag="po",
                                          name="po_ps")
                        po_pend = oproj_mms(po_ps, hi_i)
                    po_pend.pop(0)()
                if len(pend_pv) >= 2:
                    pop_pv()
                pend_pv.append((ops, h, kb, pt))
            pend_fin.append((ops, hi_i, po_ps, ti, po))
        while pend_pv:
            pop_pv()

        # o_proj tail for s-blocks 8..15: two-phase per block — the six
        # matmuls reading ot[0..2] run immediately (those columns are long
        # written), the two reading ot[3] (written by the final heads) are
        # deferred; four PSUM slots stay rotating so PE never idles
        pools = [sps_p, sps_p, ops_p, po_p]
        tags = ["sc", "sc", "ops", "po"]
        pend_stores = []

        def flush_tail():
            po_ps, sb, late, i = pend_stores.pop(0)
            for mm in late:
                mm()
            oproj_store(po_ps, sb, nc.scalar if i % 2 == 0 else nc.vector)

        for i, sb in enumerate(range(n_sb // 2, n_sb)):
            pool, tag = pools[i % 4], tags[i % 4]
            po_ps = pool.tile([128, D], F32, tag=tag, name="po_ps")
            mms = oproj_mms(po_ps, sb, t_order=(0, 1, 2, 3))
            for mm in mms[:6]:
                mm()
            pend_stores.append((po_ps, sb, mms[6:], i))
            if len(pend_stores) >= 3:
                flush_tail()
        while pend_stores:
            flush_tail()

    nc.compile()
    return nc


def _perm128():
    """[128,128] permutation: out = P.T @ x swaps 32-row halves within
    each 64-row group. P[k, m] = 1 iff k == swap(m)."""
    p = np.zeros((128, 128), np.float32)
    for m in range(128):
        k = m + 32 if (m % 64) < 32 else m - 32
        p[k, m] = 1.0
    return p


def _rope_tables(token_positions):
    pos = np.asarray(token_positions).astype(np.float32)
    half = D_K // 2
    inv_freq = (THETA ** (-np.arange(half, dtype=np.float32) * 2.0 / D_K))
    ang = pos[None, :].astype(np.float32) * inv_freq[:, None]     # [32, S]
    cos = np.cos(ang).astype(np.float32)
    sin = np.sin(ang).astype(np.float32)
    cos128 = np.tile(cos, (4, 1))                                 # [128, S]
    sin128 = np.empty((128, pos.shape[0]), np.float32)
    for g in range(4):
        sgn = -1.0 if (g % 2 == 0) else 1.0
        sin128[g * 32:(g + 1) * 32] = sgn * sin
    return np.ascontiguousarray(cos128), np.ascontiguousarray(sin128)


def kernel(x, W_qkv, W_o, token_positions):
    out, _ = _kernel_impl(x, W_qkv, W_o, token_positions, trace=False)
    return out


def _kernel_impl(x, W_qkv, W_o, token_positions, trace=False):
    global _compiled
    import ml_dtypes
    from concourse.bass_utils import run_bass_kernel_spmd

    BF = ml_dtypes.bfloat16
    x = np.asarray(x, dtype=np.float32)
    W_qkv = np.asarray(W_qkv, dtype=np.float32)
    W_o = np.asarray(W_o, dtype=np.float32)

    if _compiled is None:
        _compiled = _build_program()
    nc = _compiled

    cos128, sin128 = _rope_tables(token_positions)
    perm = np.concatenate([np.arange(0, D_K, 2), np.arange(1, D_K, 2)])

    in_maps = []
    for c in range(N_CORES):
        b, g = divmod(c, 2)
        heads = range(g * HPC, (g + 1) * HPC)
        qrows = np.concatenate(
            [W_qkv[h * D_K:(h + 1) * D_K][perm] for h in heads])
        krows = np.concatenate(
            [W_qkv[D + h * D_K:D + (h + 1) * D_K][perm] for h in heads])
        vrows = np.concatenate(
            [W_qkv[2 * D + h * D_K:2 * D + (h + 1) * D_K] for h in heads])
        wqkvt = np.ascontiguousarray(
            np.concatenate([qrows, krows, vrows]).T.astype(BF))  # [1024,1536]
        wotm = np.ascontiguousarray(
            W_o[:, g * DG:(g + 1) * DG].T.astype(BF))            # [512,1024]
        in_maps.append({
            "xt": np.ascontiguousarray(x[b].T.astype(BF)),
            "wqkvt": wqkvt,
            "wot": wotm,
            "perm": _perm128().astype(BF),
            "cost": cos128.astype(BF),
            "sint": sin128.astype(BF),
        })

    res = run_bass_kernel_spmd(nc, in_maps, list(range(N_CORES)), trace=trace)
    out = np.empty((BS, S, D), dtype=np.float32)
    for b in range(BS):
        out[b] = (res.results[2 * b]["out"].astype(np.float32) +
                  res.results[2 * b + 1]["out"].astype(np.float32))
    return out, res.exec_time_ns


# revision 53
# speedup vs baseline: 1.3046x; 1.0164x over previous
"""Multi-head causal attention with interleaved RoPE on 8 Trainium2 cores.

nn_MultiHeadAttention: x[4,2048,1024], W_qkv[3072,1024], W_o[1024,1024],
16 heads x d_k=64, interleaved RoPE, causal softmax.

Sharding: core c = 2*b + g handles batch b (of 4) and head-group g (of 2,
8 heads each). Each core computes a full-width partial output for its batch
(o_heads @ W_o[:, group-cols]); the host sums the two partials per batch
(the "all-reduce after o_proj", done on host at gather time).

Device schedule (per core), engineered against the TimelineSim cost model:
 - everything matmul-adjacent is bf16 (x, W, q_rot/k_rot, v, P, o^T): PE is
   1 cyc/row at any moving size, DMA bytes halve, and all of it stays in
   SBUF across the kernel.
 - phase 1: x strips 0,1 -> q/k projection + RoPE (rotate-half via a
   host-permuted W + perm matmul) and v projection, one pass per strip.
   Strip 0 runs db-major with 8 open PSUM groups so PE streams while the
   prologue DMAs land.
 - overlap phase: strips 2,3 are interleaved with the ENTIRE first half of
   attention (q <= 1024 only needs k rows < 1024 = strips 0,1, by
   causality). Attention here uses 512-wide q-tiles so its PSUM footprint
   (3 score slots + 2 accumulators) coexists with a 3-slot projection ring.
 - second half of attention (q2=1) is ACT(exp)-bound: QK matmuls run two
   blocks ahead of the lagged PV matmuls (pending PVs carry across head
   boundaries so the exp stream never bubbles), and o_proj for the first 8
   s-blocks rides the PE bubbles, staged through SBUF, written as bf16
   partials (the host all-reduce sums in f32).
 - o_proj tail: per s-block the six matmuls reading early-written o^T tiles
   fire immediately, the two reading the last head's tile are deferred; the
   last head's softmax-normalize is split in halves to unblock them sooner;
   four PSUM slots rotate so PE never waits on a staging copy.
"""

import numpy as np
from contextlib import ExitStack

NUM_HEADS = 16
D_K = 64
THETA = 10000.0
BS, S, D = 4, 2048, 1024
N_CORES = 8
HPC = NUM_HEADS // 2          # heads per core = 8
DG = HPC * D_K                # per-core head width = 512

_compiled = None


def _build_program():
    import concourse.bass as bass
    import concourse.mybir as mybir
    import concourse.tile as tile
    from concourse import bacc

    F32 = mybir.dt.float32
    FR = mybir.dt.float32r
    BF = mybir.dt.bfloat16
    AF = mybir.ActivationFunctionType

    nc = bacc.Bacc("TRN2", target_bir_lowering=False, debug=False,
                   num_devices=N_CORES)

    xt_d = nc.dram_tensor("xt", [D, S], BF, kind="ExternalInput")
    wqkvt_d = nc.dram_tensor("wqkvt", [D, 3 * DG], BF, kind="ExternalInput")
    wot_d = nc.dram_tensor("wot", [DG, D], BF, kind="ExternalInput")
    perm_d = nc.dram_tensor("perm", [128, 128], BF, kind="ExternalInput")
    cos_d = nc.dram_tensor("cost", [128, S], BF, kind="ExternalInput")
    sin_d = nc.dram_tensor("sint", [128, S], BF, kind="ExternalInput")
    out_d = nc.dram_tensor("out", [S, D], BF, kind="ExternalOutput")

    n_sb = S // 128           # 16 s-blocks
    n_st = S // 512           # 4 s-tiles
    n_db = D // 128           # 8 d-blocks
    inv_sqrt_dk = 1.0 / float(np.sqrt(D_K))

    with tile.TileContext(nc) as tc, ExitStack() as octx:
        OP = octx.enter_context
        # ---------- persistent pools (whole kernel) ----------
        qk_p = OP(tc.tile_pool(name="qk", bufs=1))
        qrot = [qk_p.tile([128, S], BF, tag=f"qrot{i}", name=f"qrot{i}")
                for i in range(4)]
        krot = [qk_p.tile([128, S], BF, tag=f"krot{i}", name=f"krot{i}")
                for i in range(4)]
        wot_p = OP(tc.tile_pool(name="wot", bufs=1))
        wot = [wot_p.tile([128, D], BF, tag=f"wot{i}", name=f"wott{i}")
               for i in range(4)]
        ot_p = OP(tc.tile_pool(name="ot", bufs=1))
        ot = [ot_p.tile([128, S], BF, tag=f"ot{i}", name=f"oti{i}")
              for i in range(4)]
        const_p = OP(tc.tile_pool(name="amisc", bufs=1))
        # multiplicative causal mask for the S^T diagonal block:
        # 1 where k <= q, 0 where k > q
        dmask = const_p.tile([128, 128], BF)
        nc.gpsimd.memset(dmask[:], 1.0)
        nc.gpsimd.affine_select(
            out=dmask[:], in_=dmask[:],
            compare_op=mybir.AluOpType.is_ge, fill=0.0, base=0,
            pattern=[[1, 128]], channel_multiplier=-1,
        )
        # v in [v | ones] augmented layout, bf16; ones columns set up front
        vaug_p = OP(tc.tile_pool(name="vaug", bufs=1))
        vaug = [vaug_p.tile([128, HPC * (D_K + 1)], BF, tag=f"va{i}",
                            name=f"va{i}") for i in range(n_sb)]
        for i in range(n_sb):
            nc.gpsimd.memset(
                vaug[i][:].rearrange("p (h c) -> p h c", c=D_K + 1)
                [:, :, D_K:D_K + 1], 1.0)
        pt_p = OP(tc.tile_pool(name="pt", bufs=6))
        nrm_p = OP(tc.tile_pool(name="nrm", bufs=2))
        # split-softmax: the first three second-half heads process their
        # k<1024 part inside the (PE-bound) overlap phase; partial sums
        # land here and are added to the diagonal part at drain time
        A_HEADS = [1, 0, 3]
        unnA_p = OP(tc.tile_pool(name="unnA", bufs=1))
        unnA = {h: unnA_p.tile([D_K + 1, 1024], F32, tag=f"unnA{h}",
                               name=f"unnA{h}") for h in A_HEADS}

        # ---------------- attention building blocks ----------------
        def qk_exp_mask(sc_pool, qt, ti, po, q0, kb):
            """QK matmuls + exp + diag mask for one (head, k-block) against
            q-range [q0, q0+qt); returns the bf16 probability tile."""
            c0 = max(0, kb * 128 - q0)
            sc = sc_pool.tile([128, qt], F32, tag="sc", name="sc")
            lo = c0
            while lo < qt:                      # per-512 PSUM bank chunks
                hi = min(lo - lo % 512 + 512, qt)
                nc.tensor.matmul(
                    sc[:, lo:hi],
                    krot[ti][po:po + 64, kb * 128:(kb + 1) * 128],
                    qrot[ti][po:po + 64, q0 + lo:q0 + hi],
                    start=True, stop=True)
                lo = hi
            pt = pt_p.tile([128, qt], BF, tag="pt", name="pt")
            nc.scalar.activation(pt[:, c0:qt], sc[:, c0:qt],
                                 AF.Exp, scale=inv_sqrt_dk)
            if kb * 128 >= q0:                  # causal diagonal, bf16 2x
                nc.vector.tensor_mul(pt[:, c0:c0 + 128],
                                     pt[:, c0:c0 + 128], dmask[:])
            return pt

        def emit_pv(ops, qt, h, q0, kb_end, kb, pt):
            vlo = h * (D_K + 1)
            c0 = max(0, kb * 128 - q0)
            lo = c0
            while lo < qt:
                hi = min(lo - lo % 512 + 512, qt)
                last = kb_end - 1 if hi == qt else (q0 + hi) // 128 - 1
                nc.tensor.matmul(
                    ops[:, lo:hi],
                    vaug[kb][:, vlo:vlo + D_K + 1],
                    pt[:, lo:hi],
                    start=(kb == 0), stop=(kb == last))
                lo = hi

        def normalize(ops, qt, ti, po, q0, lo=0, hi=None, drain=None,
                      addA=None):
            """Drain the PV accumulator to SBUF right away (frees the PSUM
            slot), then recip/broadcast/scale into o^T. lo/hi select a
            column sub-range; drain picks the engine for the drain copy
            (ACT when it is known-idle, e.g. the phase-boundary pair)."""
            hi = qt if hi is None else hi
            w = hi - lo
            unnorm = nrm_p.tile([D_K + 1, qt], F32, tag="unnorm",
                                name="unnorm")
            if addA is not None:
                nc.vector.tensor_add(unnorm[:, 0:w], ops[:, lo:hi],
                                     addA[:, lo:hi])
            elif drain is nc.scalar:
                nc.scalar.copy(unnorm[:, 0:w], ops[:, lo:hi])
            else:
                nc.vector.tensor_copy(unnorm[:, 0:w], ops[:, lo:hi])
            rinv = nrm_p.tile([1, qt], F32, tag="rinv", name="rinv")
            nc.vector.reciprocal(rinv[:, 0:w], unnorm[D_K:D_K + 1, 0:w])
            den = nrm_p.tile([64, qt], F32, tag="den", name="den")
            nc.gpsimd.partition_broadcast(den[:, 0:w], rinv[:, 0:w])
            if po == 0:
                nc.vector.tensor_mul(ot[ti][0:64, q0 + lo:q0 + hi],
                                     unnorm[0:D_K, 0:w], den[:, 0:w])
            else:
                onrm = nrm_p.tile([64, qt], BF, tag="onrm", name="onrm")
                nc.vector.tensor_mul(onrm[:, 0:w], unnorm[0:D_K, 0:w],
                                     den[:, 0:w])
                nc.sync.dma_start(ot[ti][64:128, q0 + lo:q0 + hi],
                                  onrm[:, 0:w])

        # ============ projection + first-half attention ============
        with ExitStack() as p1s:
            P1 = p1s.enter_context
            cs_p = P1(tc.tile_pool(name="cs", bufs=1))
            xt_p = P1(tc.tile_pool(name="xtp", bufs=2))
            w_p = P1(tc.tile_pool(name="w", bufs=1))
            tmp_p = P1(tc.tile_pool(name="tmp", bufs=5))
            rot_p = P1(tc.tile_pool(name="rot", bufs=4))

            perm_t = cs_p.tile([128, 128], BF, name="perm_t")
            cos_t = cs_p.tile([128, S], BF)
            sin_t = cs_p.tile([128, S], BF)

            def load_cs(st):
                sl = slice(st * 512, (st + 1) * 512)
                nc.scalar.dma_start(cos_t[:, sl], cos_d.ap()[:, sl])
                nc.scalar.dma_start(sin_t[:, sl], sin_d.ap()[:, sl])

            def load_xt_strip(st):
                """One batched DMA (1024 descriptors, one DGE pass)."""
                t = xt_p.tile([128, n_db * 512], BF, tag="xts", name="xts")
                src = xt_d.ap().rearrange("(db p) (st s) -> p db (st s)",
                                          p=128, st=n_st)
                nc.sync.dma_start(
                    t[:].rearrange("p (db s) -> p db s", db=n_db),
                    src[:, :, st * 512:(st + 1) * 512])
                return [t[:, db * 512:(db + 1) * 512] for db in range(n_db)]

            # W_qk^T as 8 per-db full-width tiles interleaved with the 8
            # per-db strip-0 chunks: as chunk db lands, all 8 e-blocks can
            # consume it (db-major sweep below)
            wqkf = [w_p.tile([128, 2 * DG], BF, tag=f"wqkf{i}",
                             name=f"wqkf{i}") for i in range(n_db)]
            wv_t = w_p.tile([128, n_db * DG], BF, name="wv_t")

            def wqk_slice(db, eb):
                return wqkf[db][:, eb * 128:(eb + 1) * 128]

            wv = [wv_t[:, db * DG:(db + 1) * DG] for db in range(n_db)]

            rope_pend = []

            def rope_phase2():
                """swap-matmul + t2 + add for a previous block (lag-1 so the
                perm matmul doesn't head-of-line-block the PE queue). The
                final add runs on GPSIMD: DVE is loaded during the overlap
                phase, Pool is idle."""
                pp, qtmp, t1, dst, sl = rope_pend.pop(0)
                psw = pp.tile([128, 512], F32, tag="pp", name="psw")
                nc.tensor.matmul(psw[:], perm_t[:], qtmp[:],
                                 start=True, stop=True)
                t2 = rot_p.tile([128, 512], BF, tag="t2", name="t2")
                nc.vector.tensor_mul(t2[:], psw[:], sin_t[:, sl])
                nc.gpsimd.tensor_add(dst[:, sl], t1[:], t2[:])

            def rope_tail(pp, eb, ps, sl):
                qtmp = tmp_p.tile([128, 512], BF, tag="qtmp")
                nc.scalar.copy(qtmp[:], ps[:])
                t1 = rot_p.tile([128, 512], BF, tag="t1")
                nc.vector.tensor_mul(t1[:], qtmp[:], cos_t[:, sl])
                if rope_pend:
                    rope_phase2()
                dst = qrot[eb] if eb < 4 else krot[eb - 4]
                rope_pend.append((pp, qtmp, t1, dst, sl))

            def emit_eb(pp, st, xts, eb):
                sl = slice(st * 512, (st + 1) * 512)
                ps = pp.tile([128, 512], F32, tag="pp", name="ps")
                for db in range(n_db):
                    nc.tensor.matmul(
                        ps[:], wqk_slice(db, eb), xts[db][:],
                        start=(db == 0), stop=(db == n_db - 1))
                rope_tail(pp, eb, ps, sl)

            def emit_v(pp, st, xts, j):
                sb = st * 4 + j
                ps = pp.tile([128, 512], F32, tag="pp", name="vps")
                for db in range(n_db):
                    nc.tensor.matmul(
                        ps[:], xts[db][:, j * 128:(j + 1) * 128], wv[db][:],
                        start=(db == 0), stop=(db == n_db - 1))
                src = ps[:].rearrange("p (h c) -> p h c", c=D_K)
                dst = vaug[sb][:].rearrange("p (h c) -> p h c", c=D_K + 1)
                nc.scalar.copy(dst[:, :, 0:D_K], src)

            # ---- strips 0,1: deep PSUM ring, db-major strip 0 ----
            with ExitStack() as s01:
                pp8 = s01.enter_context(
                    tc.tile_pool(name="pp8", bufs=8, space="PSUM"))
                x0_p = s01.enter_context(tc.tile_pool(name="x0p", bufs=1))
                x0 = [x0_p.tile([128, 512], BF, tag=f"x0{db}",
                                name=f"x0t{db}") for db in range(n_db)]
                for db in range(n_db):
                    nc.sync.dma_start(
                        wqkf[db][:],
                        wqkvt_d.ap()[db * 128:(db + 1) * 128, 0:1024])
                    nc.sync.dma_start(
                        x0[db][:],
                        xt_d.ap()[db * 128:(db + 1) * 128, 0:512])
                nc.scalar.dma_start(perm_t[:], perm_d.ap())
                load_cs(0)
                load_cs(1)
                xts1 = load_xt_strip(1)
                nc.sync.dma_start(
                    wv_t[:].rearrange("p (db e) -> p db e", db=n_db),
                    wqkvt_d.ap().rearrange("(db p) e -> p db e", p=128)
                    [:, :, 1024:1536])
                for t in range(4):
                    nc.scalar.dma_start(
                        wot[t][:], wot_d.ap()[t * 128:(t + 1) * 128, :])
                # strip 0, db-major: 8 open accumulation groups
                pss = [pp8.tile([128, 512], F32, tag="pp", name="pss")
                       for _ in range(8)]
                for db in range(n_db):
                    for eb in range(8):
                        nc.tensor.matmul(
                            pss[eb][:], wqk_slice(db, eb), x0[db][:],
                            start=(db == 0), stop=(db == n_db - 1))
                for eb in range(8):
                    rope_tail(pp8, eb, pss[eb], slice(0, 512))
                # strip 1 e-blocks before strip 0's v so PE doesn't wait on
                # the wv load; v projections follow once wv is resident
                load_cs(2)
                xts2 = load_xt_strip(2)
                for eb in range(8):
                    emit_eb(pp8, 1, xts1, eb)
                for j in range(4):
                    emit_v(pp8, 0, x0, j)
                for j in range(4):
                    emit_v(pp8, 1, xts1, j)
                while rope_pend:
                    rope_phase2()

            # ---- overlap: strips 2,3 interleaved with all of q2=0 ----
            # (q < 1024 attends only to k < 1024 = strips 0,1)
            with ExitStack() as ovl:
                sc0_p = ovl.enter_context(
                    tc.tile_pool(name="sc0", bufs=3, space="PSUM"))
                ops0_p = ovl.enter_context(
                    tc.tile_pool(name="ops0", bufs=2, space="PSUM"))
                pp3 = ovl.enter_context(
                    tc.tile_pool(name="pp3", bufs=3, space="PSUM"))

                load_cs(3)
                xts3 = load_xt_strip(3)
                strip_units = (
                    [lambda eb=eb: emit_eb(pp3, 2, xts2, eb)
                     for eb in range(8)] +
                    [lambda j=j: emit_v(pp3, 2, xts2, j) for j in range(4)] +
                    [lambda eb=eb: emit_eb(pp3, 3, xts3, eb)
                     for eb in range(8)] +
                    [lambda j=j: emit_v(pp3, 3, xts3, j) for j in range(4)])
                su_i = 0
                step = 0
                pace = 4

                QT = 512
                for ti in range(4):
                    for qt_i in range(2):
                        q0 = qt_i * 512
                        kb_end = (q0 + QT) // 128
                        ops2 = [ops0_p.tile([D_K + 1, QT], F32, tag="ops0",
                                            name="ops0")
                                for _ in range(2)]
                        pend_pv = []
                        for kb in range(kb_end):
                            for s in range(2):
                                pt = qk_exp_mask(sc0_p, QT, ti, s * 64,
                                                 q0, kb)
                                if len(pend_pv) >= 3:
                                    emit_pv(*pend_pv.pop(0))
                                pend_pv.append(
                                    (ops2[s], QT, 2 * ti + s, q0, kb_end,
                                     kb, pt))
                            step += 1
                            if step % pace == 0 and su_i < len(strip_units):
                                strip_units[su_i]()
                                su_i += 1
                        for a in pend_pv:
                            emit_pv(*a)
                        for s in range(2):
                            normalize(ops2[s], QT, ti, s * 64, q0,
                                      drain=(nc.scalar if ti == 3
                                             and qt_i == 1 and s == 1
                                             else None))
                # split-softmax A-halves: q in [1024,2048) x k < 1024
                # (full-height blocks: no masks, strips 0,1 only)
                pace = 7
                for h in A_HEADS:
                    tiA, poA = h // 2, (h % 2) * 64
                    for qt_i in range(2):
                        q0a = 1024 + qt_i * 512
                        opsA = ops0_p.tile([D_K + 1, QT], F32, tag="ops0",
                                           name="opsA")
                        pend_pv = []
                        for kb in range(8):
                            pt = qk_exp_mask(sc0_p, QT, tiA, poA, q0a, kb)
                            if len(pend_pv) >= 3:
                                emit_pv(*pend_pv.pop(0))
                            pend_pv.append((opsA, QT, h, q0a, 8, kb, pt))
                            step += 1
                            if step % pace == 0 and su_i < len(strip_units):
                                strip_units[su_i]()
                                su_i += 1
                        for a in pend_pv:
                            emit_pv(*a)
                        nc.vector.tensor_copy(
                            unnA[h][:, qt_i * 512:(qt_i + 1) * 512],
                            opsA[:])
                while su_i < len(strip_units):
                    strip_units[su_i]()
                    su_i += 1
                while rope_pend:
                    rope_phase2()

        # ============ second-half attention + o_proj ============
        QT2 = 1024
        sps_p = OP(tc.tile_pool(name="sps", bufs=2, space="PSUM"))
        ops_p = OP(tc.tile_pool(name="ops", bufs=1, space="PSUM"))
        po_p = OP(tc.tile_pool(name="po", bufs=1, space="PSUM"))
        outs_p = OP(tc.tile_pool(name="outs", bufs=4))

        def oproj_mms(po_ps, sb, t_order=(0, 1, 2, 3)):
            """o_proj matmuls t-major so callers can defer the tiles whose
            ot columns land last."""
            ssl = slice(sb * 128, (sb + 1) * 128)
            out = []
            for t in t_order:
                for eh in range(2):
                    esl = slice(eh * 512, (eh + 1) * 512)
                    out.append(lambda esl=esl, t=t: nc.tensor.matmul(
                        po_ps[:, esl], ot[t][:, ssl], wot[t][:, esl],
                        start=(t == t_order[0]), stop=(t == t_order[-1])))
            return out

        def oproj_store(po_ps, sb, engine):
            ostage = outs_p.tile([128, D], BF, tag="ostage", name="ostage")
            if engine is nc.scalar:
                nc.scalar.copy(ostage[:], po_ps[:])
            else:
                engine.tensor_copy(ostage[:], po_ps[:])
            nc.sync.dma_start(out_d.ap()[sb * 128:(sb + 1) * 128, :],
                              ostage[:])

        # kb visit order alternates full-height blocks (1024-wide exps) with
        # diagonal blocks (short exps) so ACT always has a long exp in
        # flight to hide the short ones' dependency latency
        kb_order = list(range(16))
        # per 512-column PSUM chunk, the first/last contributing kb in
        # emission order (start/stop accumulation flags)
        contrib = {0: [kb for kb in kb_order if max(0, kb * 128 - QT2) < 512],
                   512: kb_order[:]}
        pv_first = {lo: ks[0] for lo, ks in contrib.items()}
        pv_last = {lo: ks[-1] for lo, ks in contrib.items()}

        def emit_pv_q21(ops, h, kb, pt, k0=0):
            vlo = h * (D_K + 1)
            c0 = max(0, kb * 128 - QT2)
            for lo in (0, 512):
                if c0 >= lo + 512:
                    continue
                nc.tensor.matmul(
                    ops[:, max(c0, lo):lo + 512],
                    vaug[kb][:, vlo:vlo + D_K + 1],
                    pt[:, max(c0, lo):lo + 512],
                    start=(kb == k0), stop=(kb == pv_last[lo]))

        # within each ti, the po=64 head (whose o^T lands via DMA) runs
        # first so the final ot write before the tail is the fast DVE path.
        # The two pending PVs carry ACROSS head boundaries: the next head's
        # first QKs are emitted before the previous head's last PVs, so the
        # exp stream never sees a boundary bubble.
        h_order = [1, 0, 3, 2, 5, 4, 7, 6]
        pend_pv = []
        pend_fin = []                 # (ops, hi_i, po_ps, ti, po) to close

        def pop_pv():
            ops, h, kb, pt, k0 = pend_pv.pop(0)
            emit_pv_q21(ops, h, kb, pt, k0)
            if h == h_order[-1] and kb == pv_last[0]:
                # last head: normalize the first half as soon as its PSUM
                # chunk closes, so the tail's deferred matmuls unblock early
                normalize(ops, QT2, h // 2, (h % 2) * 64, QT2, 0, 512)
            if kb == kb_order[-1] and pend_fin:
                ops_f, hi_f, po_ps_f, ti_f, po_f, h_f = pend_fin.pop(0)
                oproj_store(po_ps_f, hi_f, nc.vector)
                if hi_f == 7:
                    normalize(ops_f, QT2, ti_f, po_f, QT2, 512, QT2)
                else:
                    normalize(ops_f, QT2, ti_f, po_f, QT2,
                              addA=unnA.get(h_f))

        for hi_i, h in enumerate(h_order):
            ti, po = h // 2, (h % 2) * 64
            split = h in A_HEADS
            kbs = kb_order[8:] if split else kb_order
            u_lo, u_hi = (0, 8) if split else (4, 12)
            k0 = 8 if split else 0
            ops = ops_p.tile([D_K + 1, QT2], F32, tag="ops", name="ops")
            po_ps = None
            po_pend = []
            for u, kb in enumerate(kbs):
                pt = qk_exp_mask(sps_p, QT2, ti, po, QT2, kb)
                # o_proj matmuls placed before the lagged PVs so the QK
                # stream stays ahead of the exp stream
                if u_lo <= u < u_hi:
                    if po_ps is None:
                        po_ps = po_p.tile([128, D], F32, tag="po",
                                          name="po_ps")
                        po_pend = oproj_mms(po_ps, hi_i)
                    po_pend.pop(0)()
                if len(pend_pv) >= 2:
                    pop_pv()
                pend_pv.append((ops, h, kb, pt, k0))
            pend_fin.append((ops, hi_i, po_ps, ti, po, h))
        while pend_pv:
            pop_pv()

        # o_proj tail for s-blocks 8..15: two-phase per block — the six
        # matmuls reading ot[0..2] run immediately (those columns are long
        # written), the two reading ot[3] (written by the final heads) are
        # deferred; four PSUM slots stay rotating so PE never idles
        pools = [sps_p, sps_p, ops_p, po_p]
        tags = ["sc", "sc", "ops", "po"]
        pend_stores = []

        def flush_tail():
            po_ps, sb, late, i = pend_stores.pop(0)
            for mm in late:
                mm()
            oproj_store(po_ps, sb, nc.scalar if i % 2 == 0 else nc.vector)

        for i, sb in enumerate(range(n_sb // 2, n_sb)):
            pool, tag = pools[i % 4], tags[i % 4]
            po_ps = pool.tile([128, D], F32, tag=tag, name="po_ps")
            mms = oproj_mms(po_ps, sb, t_order=(0, 1, 2, 3))
            for mm in mms[:6]:
                mm()
            pend_stores.append((po_ps, sb, mms[6:], i))
            if len(pend_stores) >= 2:
                flush_tail()
        while pend_stores:
            flush_tail()

    nc.compile()
    return nc


def _perm128():
    """[128,128] permutation: out = P.T @ x swaps 32-row halves within
    each 64-row group. P[k, m] = 1 iff k == swap(m)."""
    p = np.zeros((128, 128), np.float32)
    for m in range(128):
        k = m + 32 if (m % 64) < 32 else m - 32
        p[k, m] = 1.0
    return p


def _rope_tables(token_positions):
    pos = np.asarray(token_positions).astype(np.float32)
    half = D_K // 2
    inv_freq = (THETA ** (-np.arange(half, dtype=np.float32) * 2.0 / D_K))
    ang = pos[None, :].astype(np.float32) * inv_freq[:, None]     # [32, S]
    cos = np.cos(ang).astype(np.float32)
    sin = np.sin(ang).astype(np.float32)
    cos128 = np.tile(cos, (4, 1))                                 # [128, S]
    sin128 = np.empty((128, pos.shape[0]), np.float32)
    for g in range(4):
        sgn = -1.0 if (g % 2 == 0) else 1.0
        sin128[g * 32:(g + 1) * 32] = sgn * sin
    return np.ascontiguousarray(cos128), np.ascontiguousarray(sin128)


def kernel(x, W_qkv, W_o, token_positions):
    out, _ = _kernel_impl(x, W_qkv, W_o, token_positions, trace=False)
    return out


def _kernel_impl(x, W_qkv, W_o, token_positions, trace=False):
    global _compiled
    import ml_dtypes
    from concourse.bass_utils import run_bass_kernel_spmd

    BF = ml_dtypes.bfloat16
    x = np.asarray(x, dtype=np.float32)
    W_qkv = np.asarray(W_qkv, dtype=np.float32)
    W_o = np.asarray(W_o, dtype=np.float32)

    if _compiled is None:
        _compiled = _build_program()
    nc = _compiled

    cos128, sin128 = _rope_tables(token_positions)
    perm = np.concatenate([np.arange(0, D_K, 2), np.arange(1, D_K, 2)])

    in_maps = []
    for c in range(N_CORES):
        b, g = divmod(c, 2)
        heads = range(g * HPC, (g + 1) * HPC)
        qrows = np.concatenate(
            [W_qkv[h * D_K:(h + 1) * D_K][perm] for h in heads])
        krows = np.concatenate(
            [W_qkv[D + h * D_K:D + (h + 1) * D_K][perm] for h in heads])
        vrows = np.concatenate(
            [W_qkv[2 * D + h * D_K:2 * D + (h + 1) * D_K] for h in heads])
        wqkvt = np.ascontiguousarray(
            np.concatenate([qrows, krows, vrows]).T.astype(BF))  # [1024,1536]
        wotm = np.ascontiguousarray(
            W_o[:, g * DG:(g + 1) * DG].T.astype(BF))            # [512,1024]
        in_maps.append({
            "xt": np.ascontiguousarray(x[b].T.astype(BF)),
            "wqkvt": wqkvt,
            "wot": wotm,
            "perm": _perm128().astype(BF),
            "cost": cos128.astype(BF),
            "sint": sin128.astype(BF),
        })

    res = run_bass_kernel_spmd(nc, in_maps, list(range(N_CORES)), trace=trace)
    out = np.empty((BS, S, D), dtype=np.float32)
    for b in range(BS):
        out[b] = (res.results[2 * b]["out"].astype(np.float32) +
                  res.results[2 * b + 1]["out"].astype(np.float32))
    return out, res.exec_time_ns


# revision 59
# speedup vs baseline: 1.3077x; 1.0024x over previous
"""Multi-head causal attention with interleaved RoPE on 8 Trainium2 cores.

nn_MultiHeadAttention: x[4,2048,1024], W_qkv[3072,1024], W_o[1024,1024],
16 heads x d_k=64, interleaved RoPE, causal softmax.

Sharding: core c = 2*b + g handles batch b (of 4) and head-group g (of 2,
8 heads each). Each core computes a full-width partial output for its batch
(o_heads @ W_o[:, group-cols]); the host sums the two partials per batch
(the "all-reduce after o_proj", done on host at gather time).

Device schedule (per core), engineered against the TimelineSim cost model:
 - everything matmul-adjacent is bf16 (x, W, q_rot/k_rot, v, P, o^T): PE is
   1 cyc/row at any moving size, DMA bytes halve, and all of it stays in
   SBUF across the kernel.
 - phase 1: x strips 0,1 -> q/k projection + RoPE (rotate-half via a
   host-permuted W + perm matmul) and v projection, one pass per strip.
   Strip 0 runs db-major with 8 open PSUM groups so PE streams while the
   prologue DMAs land.
 - overlap phase: strips 2,3 are interleaved with the ENTIRE first half of
   attention (q <= 1024 only needs k rows < 1024 = strips 0,1, by
   causality). Attention here uses 512-wide q-tiles so its PSUM footprint
   (3 score slots + 2 accumulators) coexists with a 3-slot projection ring.
 - second half of attention (q2=1) is ACT(exp)-bound: QK matmuls run two
   blocks ahead of the lagged PV matmuls (pending PVs carry across head
   boundaries so the exp stream never bubbles), and o_proj for the first 8
   s-blocks rides the PE bubbles, staged through SBUF, written as bf16
   partials (the host all-reduce sums in f32).
 - o_proj tail: per s-block the six matmuls reading early-written o^T tiles
   fire immediately, the two reading the last head's tile are deferred; the
   last head's softmax-normalize is split in halves to unblock them sooner;
   four PSUM slots rotate so PE never waits on a staging copy.
"""

import numpy as np
from contextlib import ExitStack

NUM_HEADS = 16
D_K = 64
THETA = 10000.0
BS, S, D = 4, 2048, 1024
N_CORES = 8
HPC = NUM_HEADS // 2          # heads per core = 8
DG = HPC * D_K                # per-core head width = 512

_compiled = None


def _build_program():
    import concourse.bass as bass
    import concourse.mybir as mybir
    import concourse.tile as tile
    from concourse import bacc

    F32 = mybir.dt.float32
    FR = mybir.dt.float32r
    BF = mybir.dt.bfloat16
    AF = mybir.ActivationFunctionType

    nc = bacc.Bacc("TRN2", target_bir_lowering=False, debug=False,
                   num_devices=N_CORES)

    xt_d = nc.dram_tensor("xt", [D, S], BF, kind="ExternalInput")
    wqkvt_d = nc.dram_tensor("wqkvt", [D, 3 * DG], BF, kind="ExternalInput")
    wot_d = nc.dram_tensor("wot", [DG, D], BF, kind="ExternalInput")
    perm_d = nc.dram_tensor("perm", [128, 128], BF, kind="ExternalInput")
    cos_d = nc.dram_tensor("cost", [128, S], BF, kind="ExternalInput")
    sin_d = nc.dram_tensor("sint", [128, S], BF, kind="ExternalInput")
    out_d = nc.dram_tensor("out", [S, D], BF, kind="ExternalOutput")

    n_sb = S // 128           # 16 s-blocks
    n_st = S // 512           # 4 s-tiles
    n_db = D // 128           # 8 d-blocks
    inv_sqrt_dk = 1.0 / float(np.sqrt(D_K))

    with tile.TileContext(nc) as tc, ExitStack() as octx:
        OP = octx.enter_context
        # ---------- persistent pools (whole kernel) ----------
        qk_p = OP(tc.tile_pool(name="qk", bufs=1))
        qrot = [qk_p.tile([128, S], BF, tag=f"qrot{i}", name=f"qrot{i}")
                for i in range(4)]
        krot = [qk_p.tile([128, S], BF, tag=f"krot{i}", name=f"krot{i}")
                for i in range(4)]
        wot_p = OP(tc.tile_pool(name="wot", bufs=1))
        wot = [wot_p.tile([128, D], BF, tag=f"wot{i}", name=f"wott{i}")
               for i in range(4)]
        ot_p = OP(tc.tile_pool(name="ot", bufs=1))
        ot = [ot_p.tile([128, S], BF, tag=f"ot{i}", name=f"oti{i}")
              for i in range(4)]
        const_p = OP(tc.tile_pool(name="amisc", bufs=1))
        # multiplicative causal mask for the S^T diagonal block:
        # 1 where k <= q, 0 where k > q
        dmask = const_p.tile([128, 128], BF)
        nc.gpsimd.memset(dmask[:], 1.0)
        nc.gpsimd.affine_select(
            out=dmask[:], in_=dmask[:],
            compare_op=mybir.AluOpType.is_ge, fill=0.0, base=0,
            pattern=[[1, 128]], channel_multiplier=-1,
        )
        # v in [v | ones] augmented layout, bf16; ones columns set up front
        vaug_p = OP(tc.tile_pool(name="vaug", bufs=1))
        vaug = [vaug_p.tile([128, HPC * (D_K + 1)], BF, tag=f"va{i}",
                            name=f"va{i}") for i in range(n_sb)]
        for i in range(n_sb):
            nc.gpsimd.memset(
                vaug[i][:].rearrange("p (h c) -> p h c", c=D_K + 1)
                [:, :, D_K:D_K + 1], 1.0)
        pt_p = OP(tc.tile_pool(name="pt", bufs=7))
        nrm_p = OP(tc.tile_pool(name="nrm", bufs=2))
        # split-softmax: the first three second-half heads process their
        # k<1024 part inside the (PE-bound) overlap phase; partial sums
        # land here and are added to the diagonal part at drain time
        A_HEADS = [1, 0, 3]
        unnA_p = OP(tc.tile_pool(name="unnA", bufs=1))
        unnA = {h: unnA_p.tile([D_K + 1, 1024], F32, tag=f"unnA{h}",
                               name=f"unnA{h}") for h in A_HEADS}

        # ---------------- attention building blocks ----------------
        def qk_exp_mask(sc_pool, qt, ti, po, q0, kb):
            """QK matmuls + exp + diag mask for one (head, k-block) against
            q-range [q0, q0+qt); returns the bf16 probability tile."""
            c0 = max(0, kb * 128 - q0)
            sc = sc_pool.tile([128, qt], F32, tag="sc", name="sc")
            lo = c0
            while lo < qt:                      # per-512 PSUM bank chunks
                hi = min(lo - lo % 512 + 512, qt)
                nc.tensor.matmul(
                    sc[:, lo:hi],
                    krot[ti][po:po + 64, kb * 128:(kb + 1) * 128],
                    qrot[ti][po:po + 64, q0 + lo:q0 + hi],
                    start=True, stop=True)
                lo = hi
            pt = pt_p.tile([128, qt], BF, tag="pt", name="pt")
            nc.scalar.activation(pt[:, c0:qt], sc[:, c0:qt],
                                 AF.Exp, scale=inv_sqrt_dk)
            if kb * 128 >= q0:                  # causal diagonal, bf16 2x
                nc.vector.tensor_mul(pt[:, c0:c0 + 128],
                                     pt[:, c0:c0 + 128], dmask[:])
            return pt

        def emit_pv(ops, qt, h, q0, kb_end, kb, pt):
            vlo = h * (D_K + 1)
            c0 = max(0, kb * 128 - q0)
            lo = c0
            while lo < qt:
                hi = min(lo - lo % 512 + 512, qt)
                last = kb_end - 1 if hi == qt else (q0 + hi) // 128 - 1
                nc.tensor.matmul(
                    ops[:, lo:hi],
                    vaug[kb][:, vlo:vlo + D_K + 1],
                    pt[:, lo:hi],
                    start=(kb == 0), stop=(kb == last))
                lo = hi

        def normalize(ops, qt, ti, po, q0, lo=0, hi=None, drain=None,
                      addA=None):
            """Drain the PV accumulator to SBUF right away (frees the PSUM
            slot), then recip/broadcast/scale into o^T. lo/hi select a
            column sub-range; drain picks the engine for the drain copy
            (ACT when it is known-idle, e.g. the phase-boundary pair)."""
            hi = qt if hi is None else hi
            w = hi - lo
            unnorm = nrm_p.tile([D_K + 1, qt], F32, tag="unnorm",
                                name="unnorm")
            if addA is not None:
                nc.vector.tensor_add(unnorm[:, 0:w], ops[:, lo:hi],
                                     addA[:, lo:hi])
            elif drain is nc.scalar:
                nc.scalar.copy(unnorm[:, 0:w], ops[:, lo:hi])
            else:
                nc.vector.tensor_copy(unnorm[:, 0:w], ops[:, lo:hi])
            rinv = nrm_p.tile([1, qt], F32, tag="rinv", name="rinv")
            nc.vector.reciprocal(rinv[:, 0:w], unnorm[D_K:D_K + 1, 0:w])
            den = nrm_p.tile([64, qt], F32, tag="den", name="den")
            nc.gpsimd.partition_broadcast(den[:, 0:w], rinv[:, 0:w])
            if po == 0:
                nc.vector.tensor_mul(ot[ti][0:64, q0 + lo:q0 + hi],
                                     unnorm[0:D_K, 0:w], den[:, 0:w])
            else:
                onrm = nrm_p.tile([64, qt], BF, tag="onrm", name="onrm")
                nc.vector.tensor_mul(onrm[:, 0:w], unnorm[0:D_K, 0:w],
                                     den[:, 0:w])
                nc.sync.dma_start(ot[ti][64:128, q0 + lo:q0 + hi],
                                  onrm[:, 0:w])

        # ============ projection + first-half attention ============
        with ExitStack() as p1s:
            P1 = p1s.enter_context
            cs_p = P1(tc.tile_pool(name="cs", bufs=1))
            xt_p = P1(tc.tile_pool(name="xtp", bufs=2))
            w_p = P1(tc.tile_pool(name="w", bufs=1))
            tmp_p = P1(tc.tile_pool(name="tmp", bufs=5))
            rot_p = P1(tc.tile_pool(name="rot", bufs=4))

            perm_t = cs_p.tile([128, 128], BF, name="perm_t")
            cos_t = cs_p.tile([128, S], BF)
            sin_t = cs_p.tile([128, S], BF)

            def load_cs(st):
                sl = slice(st * 512, (st + 1) * 512)
                nc.scalar.dma_start(cos_t[:, sl], cos_d.ap()[:, sl])
                nc.scalar.dma_start(sin_t[:, sl], sin_d.ap()[:, sl])

            def load_xt_strip(st):
                """One batched DMA (1024 descriptors, one DGE pass)."""
                t = xt_p.tile([128, n_db * 512], BF, tag="xts", name="xts")
                src = xt_d.ap().rearrange("(db p) (st s) -> p db (st s)",
                                          p=128, st=n_st)
                nc.sync.dma_start(
                    t[:].rearrange("p (db s) -> p db s", db=n_db),
                    src[:, :, st * 512:(st + 1) * 512])
                return [t[:, db * 512:(db + 1) * 512] for db in range(n_db)]

            # W_qk^T as 8 per-db full-width tiles interleaved with the 8
            # per-db strip-0 chunks: as chunk db lands, all 8 e-blocks can
            # consume it (db-major sweep below)
            wqkf = [w_p.tile([128, 2 * DG], BF, tag=f"wqkf{i}",
                             name=f"wqkf{i}") for i in range(n_db)]
            wv_t = w_p.tile([128, n_db * DG], BF, name="wv_t")

            def wqk_slice(db, eb):
                return wqkf[db][:, eb * 128:(eb + 1) * 128]

            wv = [wv_t[:, db * DG:(db + 1) * DG] for db in range(n_db)]

            rope_pend = []

            def rope_phase2():
                """swap-matmul + t2 + add for a previous block (lag-1 so the
                perm matmul doesn't head-of-line-block the PE queue). The
                final add runs on GPSIMD: DVE is loaded during the overlap
                phase, Pool is idle."""
                pp, qtmp, t1, dst, sl = rope_pend.pop(0)
                psw = pp.tile([128, 512], F32, tag="pp", name="psw")
                nc.tensor.matmul(psw[:], perm_t[:], qtmp[:],
                                 start=True, stop=True)
                t2 = rot_p.tile([128, 512], BF, tag="t2", name="t2")
                nc.vector.tensor_mul(t2[:], psw[:], sin_t[:, sl])
                nc.gpsimd.tensor_add(dst[:, sl], t1[:], t2[:])

            def rope_tail(pp, eb, ps, sl):
                qtmp = tmp_p.tile([128, 512], BF, tag="qtmp")
                nc.scalar.copy(qtmp[:], ps[:])
                t1 = rot_p.tile([128, 512], BF, tag="t1")
                nc.vector.tensor_mul(t1[:], qtmp[:], cos_t[:, sl])
                if rope_pend:
                    rope_phase2()
                dst = qrot[eb] if eb < 4 else krot[eb - 4]
                rope_pend.append((pp, qtmp, t1, dst, sl))

            def emit_eb(pp, st, xts, eb):
                sl = slice(st * 512, (st + 1) * 512)
                ps = pp.tile([128, 512], F32, tag="pp", name="ps")
                for db in range(n_db):
                    nc.tensor.matmul(
                        ps[:], wqk_slice(db, eb), xts[db][:],
                        start=(db == 0), stop=(db == n_db - 1))
                rope_tail(pp, eb, ps, sl)

            def emit_v(pp, st, xts, j):
                sb = st * 4 + j
                ps = pp.tile([128, 512], F32, tag="pp", name="vps")
                for db in range(n_db):
                    nc.tensor.matmul(
                        ps[:], xts[db][:, j * 128:(j + 1) * 128], wv[db][:],
                        start=(db == 0), stop=(db == n_db - 1))
                src = ps[:].rearrange("p (h c) -> p h c", c=D_K)
                dst = vaug[sb][:].rearrange("p (h c) -> p h c", c=D_K + 1)
                nc.scalar.copy(dst[:, :, 0:D_K], src)

            # ---- strips 0,1: deep PSUM ring, db-major strip 0 ----
            with ExitStack() as s01:
                pp8 = s01.enter_context(
                    tc.tile_pool(name="pp8", bufs=8, space="PSUM"))
                x0_p = s01.enter_context(tc.tile_pool(name="x0p", bufs=1))
                x0 = [x0_p.tile([128, 512], BF, tag=f"x0{db}",
                                name=f"x0t{db}") for db in range(n_db)]
                for db in range(n_db):
                    nc.sync.dma_start(
                        wqkf[db][:],
                        wqkvt_d.ap()[db * 128:(db + 1) * 128, 0:1024])
                    nc.sync.dma_start(
                        x0[db][:],
                        xt_d.ap()[db * 128:(db + 1) * 128, 0:512])
                nc.scalar.dma_start(perm_t[:], perm_d.ap())
                load_cs(0)
                load_cs(1)
                xts1 = load_xt_strip(1)
                nc.sync.dma_start(
                    wv_t[:].rearrange("p (db e) -> p db e", db=n_db),
                    wqkvt_d.ap().rearrange("(db p) e -> p db e", p=128)
                    [:, :, 1024:1536])
                for t in range(4):
                    nc.scalar.dma_start(
                        wot[t][:], wot_d.ap()[t * 128:(t + 1) * 128, :])
                # strip 0, db-major: 8 open accumulation groups
                pss = [pp8.tile([128, 512], F32, tag="pp", name="pss")
                       for _ in range(8)]
                for db in range(n_db):
                    for eb in range(8):
                        nc.tensor.matmul(
                            pss[eb][:], wqk_slice(db, eb), x0[db][:],
                            start=(db == 0), stop=(db == n_db - 1))
                for eb in range(8):
                    rope_tail(pp8, eb, pss[eb], slice(0, 512))
                # strip 1 e-blocks before strip 0's v so PE doesn't wait on
                # the wv load; v projections follow once wv is resident
                load_cs(2)
                xts2 = load_xt_strip(2)
                for eb in range(8):
                    emit_eb(pp8, 1, xts1, eb)
                for j in range(4):
                    emit_v(pp8, 0, x0, j)
                for j in range(4):
                    emit_v(pp8, 1, xts1, j)
                while rope_pend:
                    rope_phase2()

            # ---- overlap: strips 2,3 interleaved with all of q2=0 ----
            # (q < 1024 attends only to k < 1024 = strips 0,1)
            with ExitStack() as ovl:
                sc0_p = ovl.enter_context(
                    tc.tile_pool(name="sc0", bufs=3, space="PSUM"))
                ops0_p = ovl.enter_context(
                    tc.tile_pool(name="ops0", bufs=2, space="PSUM"))
                pp3 = ovl.enter_context(
                    tc.tile_pool(name="pp3", bufs=3, space="PSUM"))

                load_cs(3)
                xts3 = load_xt_strip(3)
                strip_units = (
                    [lambda eb=eb: emit_eb(pp3, 2, xts2, eb)
                     for eb in range(8)] +
                    [lambda j=j: emit_v(pp3, 2, xts2, j) for j in range(4)] +
                    [lambda eb=eb: emit_eb(pp3, 3, xts3, eb)
                     for eb in range(8)] +
                    [lambda j=j: emit_v(pp3, 3, xts3, j) for j in range(4)])
                su_i = 0
                step = 0
                pace = 4

                QT = 512
                for ti in range(4):
                    for qt_i in range(2):
                        q0 = qt_i * 512
                        kb_end = (q0 + QT) // 128
                        ops2 = [ops0_p.tile([D_K + 1, QT], F32, tag="ops0",
                                            name="ops0")
                                for _ in range(2)]
                        pend_pv = []
                        for kb in range(kb_end):
                            for s in range(2):
                                pt = qk_exp_mask(sc0_p, QT, ti, s * 64,
                                                 q0, kb)
                                if len(pend_pv) >= 3:
                                    emit_pv(*pend_pv.pop(0))
                                pend_pv.append(
                                    (ops2[s], QT, 2 * ti + s, q0, kb_end,
                                     kb, pt))
                            step += 1
                            if step % pace == 0 and su_i < len(strip_units):
                                strip_units[su_i]()
                                su_i += 1
                        for a in pend_pv:
                            emit_pv(*a)
                        for s in range(2):
                            normalize(ops2[s], QT, ti, s * 64, q0,
                                      drain=(nc.scalar if ti == 3
                                             and qt_i == 1 and s == 1
                                             else None))
                # split-softmax A-halves: q in [1024,2048) x k < 1024
                # (full-height blocks: no masks, strips 0,1 only)
                pace = 7
                for h in A_HEADS:
                    tiA, poA = h // 2, (h % 2) * 64
                    for qt_i in range(2):
                        q0a = 1024 + qt_i * 512
                        opsA = ops0_p.tile([D_K + 1, QT], F32, tag="ops0",
                                           name="opsA")
                        pend_pv = []
                        for kb in range(8):
                            pt = qk_exp_mask(sc0_p, QT, tiA, poA, q0a, kb)
                            if len(pend_pv) >= 3:
                                emit_pv(*pend_pv.pop(0))
                            pend_pv.append((opsA, QT, h, q0a, 8, kb, pt))
                            step += 1
                            if step % pace == 0 and su_i < len(strip_units):
                                strip_units[su_i]()
                                su_i += 1
                        for a in pend_pv:
                            emit_pv(*a)
                        nc.vector.tensor_copy(
                            unnA[h][:, qt_i * 512:(qt_i + 1) * 512],
                            opsA[:])
                while su_i < len(strip_units):
                    strip_units[su_i]()
                    su_i += 1
                while rope_pend:
                    rope_phase2()

        # ============ second-half attention + o_proj ============
        QT2 = 1024
        sps_p = OP(tc.tile_pool(name="sps", bufs=2, space="PSUM"))
        ops_p = OP(tc.tile_pool(name="ops", bufs=1, space="PSUM"))
        po_p = OP(tc.tile_pool(name="po", bufs=1, space="PSUM"))
        outs_p = OP(tc.tile_pool(name="outs", bufs=4))

        def oproj_mms(po_ps, sb, t_order=(0, 1, 2, 3)):
            """o_proj matmuls t-major so callers can defer the tiles whose
            ot columns land last."""
            ssl = slice(sb * 128, (sb + 1) * 128)
            out = []
            for t in t_order:
                for eh in range(2):
                    esl = slice(eh * 512, (eh + 1) * 512)
                    out.append(lambda esl=esl, t=t: nc.tensor.matmul(
                        po_ps[:, esl], ot[t][:, ssl], wot[t][:, esl],
                        start=(t == t_order[0]), stop=(t == t_order[-1])))
            return out

        def oproj_store(po_ps, sb, engine):
            ostage = outs_p.tile([128, D], BF, tag="ostage", name="ostage")
            if engine is nc.scalar:
                nc.scalar.copy(ostage[:], po_ps[:])
            else:
                engine.tensor_copy(ostage[:], po_ps[:])
            nc.sync.dma_start(out_d.ap()[sb * 128:(sb + 1) * 128, :],
                              ostage[:])

        # kb visit order alternates full-height blocks (1024-wide exps) with
        # diagonal blocks (short exps) so ACT always has a long exp in
        # flight to hide the short ones' dependency latency
        kb_order = list(range(16))
        # per 512-column PSUM chunk, the first/last contributing kb in
        # emission order (start/stop accumulation flags)
        contrib = {0: [kb for kb in kb_order if max(0, kb * 128 - QT2) < 512],
                   512: kb_order[:]}
        pv_first = {lo: ks[0] for lo, ks in contrib.items()}
        pv_last = {lo: ks[-1] for lo, ks in contrib.items()}

        def emit_pv_q21(ops, h, kb, pt, k0=0):
            vlo = h * (D_K + 1)
            c0 = max(0, kb * 128 - QT2)
            for lo in (0, 512):
                if c0 >= lo + 512:
                    continue
                nc.tensor.matmul(
                    ops[:, max(c0, lo):lo + 512],
                    vaug[kb][:, vlo:vlo + D_K + 1],
                    pt[:, max(c0, lo):lo + 512],
                    start=(kb == k0), stop=(kb == pv_last[lo]))

        # within each ti, the po=64 head (whose o^T lands via DMA) runs
        # first so the final ot write before the tail is the fast DVE path.
        # The two pending PVs carry ACROSS head boundaries: the next head's
        # first QKs are emitted before the previous head's last PVs, so the
        # exp stream never sees a boundary bubble.
        h_order = [1, 0, 3, 2, 5, 4, 7, 6]
        pend_pv = []
        pend_fin = []                 # (ops, hi_i, po_ps, ti, po) to close

        def pop_pv():
            ops, h, kb, pt, k0 = pend_pv.pop(0)
            emit_pv_q21(ops, h, kb, pt, k0)
            if h == h_order[-1] and kb == pv_last[0]:
                # last head: normalize the first half as soon as its PSUM
                # chunk closes, so the tail's deferred matmuls unblock early
                normalize(ops, QT2, h // 2, (h % 2) * 64, QT2, 0, 512)
            if kb == kb_order[-1] and pend_fin:
                ops_f, hi_f, po_ps_f, ti_f, po_f, h_f = pend_fin.pop(0)
                oproj_store(po_ps_f, hi_f, nc.vector)
                if hi_f == 7:
                    normalize(ops_f, QT2, ti_f, po_f, QT2, 512, QT2)
                else:
                    normalize(ops_f, QT2, ti_f, po_f, QT2,
                              addA=unnA.get(h_f))

        for hi_i, h in enumerate(h_order):
            ti, po = h // 2, (h % 2) * 64
            split = h in A_HEADS
            kbs = kb_order[8:] if split else kb_order
            u_lo, u_hi = (0, 8) if split else (4, 12)
            k0 = 8 if split else 0
            ops = ops_p.tile([D_K + 1, QT2], F32, tag="ops", name="ops")
            po_ps = None
            po_pend = []
            for u, kb in enumerate(kbs):
                pt = qk_exp_mask(sps_p, QT2, ti, po, QT2, kb)
                # o_proj matmuls placed before the lagged PVs so the QK
                # stream stays ahead of the exp stream
                if u_lo <= u < u_hi:
                    if po_ps is None:
                        po_ps = po_p.tile([128, D], F32, tag="po",
                                          name="po_ps")
                        po_pend = oproj_mms(po_ps, hi_i)
                    po_pend.pop(0)()
                if len(pend_pv) >= 2:
                    pop_pv()
                pend_pv.append((ops, h, kb, pt, k0))
            pend_fin.append((ops, hi_i, po_ps, ti, po, h))
        while pend_pv:
            pop_pv()

        # o_proj tail for s-blocks 8..15: two-phase per block — the six
        # matmuls reading ot[0..2] run immediately (those columns are long
        # written), the two reading ot[3] (written by the final heads) are
        # deferred; four PSUM slots stay rotating so PE never idles
        pools = [sps_p, sps_p, ops_p, po_p]
        tags = ["sc", "sc", "ops", "po"]
        pend_stores = []

        def flush_tail():
            po_ps, sb, late, i = pend_stores.pop(0)
            for mm in late:
                mm()
            oproj_store(po_ps, sb, nc.scalar if i % 2 == 0 else nc.vector)

        for i, sb in enumerate(range(n_sb // 2, n_sb)):
            pool, tag = pools[i % 4], tags[i % 4]
            po_ps = pool.tile([128, D], F32, tag=tag, name="po_ps")
            mms = oproj_mms(po_ps, sb, t_order=(0, 1, 2, 3))
            for mm in mms[:6]:
                mm()
            pend_stores.append((po_ps, sb, mms[6:], i))
            if len(pend_stores) >= 2:
                flush_tail()
        while pend_stores:
            flush_tail()

    nc.compile()
    return nc


def _perm128():
    """[128,128] permutation: out = P.T @ x swaps 32-row halves within
    each 64-row group. P[k, m] = 1 iff k == swap(m)."""
    p = np.zeros((128, 128), np.float32)
    for m in range(128):
        k = m + 32 if (m % 64) < 32 else m - 32
        p[k, m] = 1.0
    return p


def _rope_tables(token_positions):
    pos = np.asarray(token_positions).astype(np.float32)
    half = D_K // 2
    inv_freq = (THETA ** (-np.arange(half, dtype=np.float32) * 2.0 / D_K))
    ang = pos[None, :].astype(np.float32) * inv_freq[:, None]     # [32, S]
    cos = np.cos(ang).astype(np.float32)
    sin = np.sin(ang).astype(np.float32)
    cos128 = np.tile(cos, (4, 1))                                 # [128, S]
    sin128 = np.empty((128, pos.shape[0]), np.float32)
    for g in range(4):
        sgn = -1.0 if (g % 2 == 0) else 1.0
        sin128[g * 32:(g + 1) * 32] = sgn * sin
    return np.ascontiguousarray(cos128), np.ascontiguousarray(sin128)


def kernel(x, W_qkv, W_o, token_positions):
    out, _ = _kernel_impl(x, W_qkv, W_o, token_positions, trace=False)
    return out


def _kernel_impl(x, W_qkv, W_o, token_positions, trace=False):
    global _compiled
    import ml_dtypes
    from concourse.bass_utils import run_bass_kernel_spmd

    BF = ml_dtypes.bfloat16
    x = np.asarray(x, dtype=np.float32)
    W_qkv = np.asarray(W_qkv, dtype=np.float32)
    W_o = np.asarray(W_o, dtype=np.float32)

    if _compiled is None:
        _compiled = _build_program()
    nc = _compiled

    cos128, sin128 = _rope_tables(token_positions)
    perm = np.concatenate([np.arange(0, D_K, 2), np.arange(1, D_K, 2)])

    in_maps = []
    for c in range(N_CORES):
        b, g = divmod(c, 2)
        heads = range(g * HPC, (g + 1) * HPC)
        qrows = np.concatenate(
            [W_qkv[h * D_K:(h + 1) * D_K][perm] for h in heads])
        krows = np.concatenate(
            [W_qkv[D + h * D_K:D + (h + 1) * D_K][perm] for h in heads])
        vrows = np.concatenate(
            [W_qkv[2 * D + h * D_K:2 * D + (h + 1) * D_K] for h in heads])
        wqkvt = np.ascontiguousarray(
            np.concatenate([qrows, krows, vrows]).T.astype(BF))  # [1024,1536]
        wotm = np.ascontiguousarray(
            W_o[:, g * DG:(g + 1) * DG].T.astype(BF))            # [512,1024]
        in_maps.append({
            "xt": np.ascontiguousarray(x[b].T.astype(BF)),
            "wqkvt": wqkvt,
            "wot": wotm,
            "perm": _perm128().astype(BF),
            "cost": cos128.astype(BF),
            "sint": sin128.astype(BF),
        })

    res = run_bass_kernel_spmd(nc, in_maps, list(range(N_CORES)), trace=trace)
    out = np.empty((BS, S, D), dtype=np.float32)
    for b in range(BS):
        out[b] = (res.results[2 * b]["out"].astype(np.float32) +
                  res.results[2 * b + 1]["out"].astype(np.float32))
    return out, res.exec_time_ns
